# revision 24
# baseline (speedup 1.0000x reference)
"""Trainium2 Bass kernel for nn_MetaNetLinearizedModel (8-core SPMD).

Math: func0 takes the patch-mean immediately after the first affine map, so
the whole per-patch computation collapses to the patch-mean vector xbar:
    f  = xbar @ Wp + bp          (xbar = patches.mean(axis=0))
    z1 = f @ W1 + b1 ; a = relu(z1) ; base = a @ W2 + b2
    coefs c[b,t,p] from MetaNet(base)
JVP term (per sample b), using linearity of the task-vector sums:
    df  = sum_t c0 * (xbar @ dWp[t]) + sum_t c1 * dbp[t]
    dz1 = df @ W1 + sum_t c2 * (f @ dW1[t]) + sum_t c3 * db1[t]
    da  = (z1 > 0) * dz1
    out = base + da @ W2 + sum_t c4 * (a @ dW2[t]) + sum_t c5 * db2[t]

Sharding (core i of 8):
  - batch slice 4i:4i+4 of x for the patch-mean (AllGather -> full xbar)
  - H-slice 384i:384(i+1) of W1/W2 for base fwd + tail (partials AllReduced /
    ReduceScattered)
  - task contraction slices of the delta tensors: dW1[:, :, Hslice],
    dW2[:, Hslice, :], dWp[:, :, Dchunk] so each core reads 1/8 of the
    deltas; the per-(b,t) coefficient scaling is folded into 8 scaled copies
    of the rhs activations and the task sum K-accumulates in PSUM.
Everything computed in transposed layout: features on partitions, batch (32)
on the free dim, so weights act as the stationary matmul operand in their
native [K, M] layout.  Matmul operands are fp16 (cast in-flight by gpsimd
DMAs); accumulation is fp32 in PSUM; the patch-mean pooling is fp32.
"""

import numpy as np

import concourse.bacc as bacc
import concourse.mybir as mybir
import concourse.tile as tile
from concourse.bass_utils import run_bass_kernel_spmd

F32 = mybir.dt.float32
F16 = mybir.dt.float16

NCORES = 8
B = 32          # batch
BL = B // NCORES  # local batch = 4
D = 768
H = 3072
T = 8
MH = 192        # metanet hidden
HS = H // NCORES   # 384 H-slice
DS = D // NCORES   # 96  D-chunk
NP = 196        # patches

# permutation of metanet output columns: p-major, even p blocks first so the
# scale rows (p in {0,2,4}) are contiguous, then the bias rows (p in {1,3,5}).
_PORDER = [0, 2, 4, 1, 3, 5]


def _metanet_perm():
    cols = []
    for p in _PORDER:
        for t in range(T):
            cols.append(t * 6 + p)
    return np.array(cols, dtype=np.int64)


def _build_nc():
    nc = bacc.Bacc("TRN2", target_bir_lowering=False, debug=False,
                   num_devices=NCORES)

    def inp(name, shape):
        return nc.dram_tensor(name, list(shape), F32, kind="ExternalInput")

    xs = inp("xs", [168, 3584])        # local 4 samples, [ (b c pi), (i pj j) ]
    selA = inp("selA", [126, 12])
    selB = inp("selB", [42, 12])
    ones = inp("ones", [1, 32])
    Wp = inp("Wp", [D, D])
    bpr = inp("bpr", [1, D])
    W1s = inp("W1s", [D, HS])
    b1r = inp("b1r", [1, HS])
    W2s = inp("W2s", [HS, D])
    mW1 = inp("mW1", [D, MH])
    mb1r = inp("mb1r", [1, MH])
    mW2p = inp("mW2p", [MH, 48])
    mb2p = inp("mb2p", [1, 48])
    b2t = inp("b2t", [128, 6])         # b2 as [128, 6] (col = k-tile)
    b2cc = inp("b2cc", [DS, 1])        # b2 chunk, per-partition scalar
    dWps = inp("dWps", [T * D, DS])    # dWp[:, :, dchunk]
    dW1s = inp("dW1s", [T * D, HS])    # dW1[:, :, hslice]
    dW2s = inp("dW2s", [T * HS, D])    # dW2[:, hslice, :]
    dbps = inp("dbps", [T, DS])
    db1s = inp("db1s", [T, HS])
    db2c = inp("db2c", [T, DS])
    bsel = inp("bsel", [128, B])       # 1.0 at this core's batch columns

    out = nc.dram_tensor("out", [DS, B], F32, kind="ExternalOutput")

    RG = [list(range(NCORES))]
    ADD = mybir.AluOpType.add
    BYP = mybir.AluOpType.bypass
    MULT = mybir.AluOpType.mult

    with tile.TileContext(nc) as tc:
        with tc.tile_pool(name="sb", bufs=1) as sb, \
             tc.tile_pool(name="ps", bufs=8, space="PSUM") as ps, \
             tc.tile_pool(name="dram", bufs=1, space="DRAM") as dr:

            def pst(p=128):
                return ps.tile([p, 32], F32, tag="ps", name="pst")

            # ---------- small/param DMAs (phase 1 needs) ----------
            selA_sb = sb.tile([126, 12], F32)
            selB_sb = sb.tile([42, 12], F32)
            ones_sb = sb.tile([1, 32], F16)
            nc.sync.dma_start(selA_sb[:], selA[:, :])
            nc.sync.dma_start(selB_sb[:], selB[:, :])
            nc.gpsimd.dma_start(ones_sb[:], ones[:, :])

            xa = sb.tile([126, 3584], F32)
            xb = sb.tile([42, 3584], F32)
            xa_dma = nc.sync.dma_start(xa[:], xs[0:126, :])
            xb_dma = nc.sync.dma_start(xb[:], xs[126:168, :])

            wp_sb = sb.tile([128, 6 * D], F16)
            wp_dma = nc.gpsimd.dma_start(
                wp_sb[:].rearrange("p (k m) -> p k m", k=6),
                Wp[:, :].rearrange("(k p) m -> p k m", k=6, p=128))
            # Gate the (FIFO) gpsimd prefetch stream behind the x tiles so the
            # pooling input doesn't contend with 35MB of weight prefetch.
            tile.add_dep_helper(wp_dma.ins, xa_dma.ins, sync=True,
                                reason="x before weight prefetch")
            tile.add_dep_helper(wp_dma.ins, xb_dma.ins, sync=True,
                                reason="x before weight prefetch")
            bpr_sb = sb.tile([1, D], F16)
            nc.gpsimd.dma_start(bpr_sb[:], bpr[:, :])

            w1_sb = sb.tile([128, 6 * HS], F16)
            nc.gpsimd.dma_start(
                w1_sb[:].rearrange("p (k m) -> p k m", k=6),
                W1s[:, :].rearrange("(k p) m -> p k m", k=6, p=128))
            b1r_sb = sb.tile([1, HS], F16)
            nc.gpsimd.dma_start(b1r_sb[:], b1r[:, :])

            w2_sb = sb.tile([128, 3 * D], F16)
            nc.gpsimd.dma_start(
                w2_sb[:].rearrange("p (k m) -> p k m", k=3),
                W2s[:, :].rearrange("(k p) m -> p k m", k=3, p=128))

            mw1_sb = sb.tile([128, 6 * MH], F16)
            nc.gpsimd.dma_start(
                mw1_sb[:].rearrange("p (k m) -> p k m", k=6),
                mW1[:, :].rearrange("(k p) m -> p k m", k=6, p=128))
            mb1r_sb = sb.tile([1, MH], F16)
            nc.gpsimd.dma_start(mb1r_sb[:], mb1r[:, :])
            mw2_sb = sb.tile([128, 96], F16)
            nc.gpsimd.dma_start(mw2_sb[:, 0:48], mW2p[0:128, :])
            nc.gpsimd.dma_start(mw2_sb[0:64, 48:96], mW2p[128:192, :])
            mb2p_sb = sb.tile([1, 48], F16)
            nc.gpsimd.dma_start(mb2p_sb[:], mb2p[:, :])
            b2t_sb = sb.tile([128, 6], F16)
            nc.gpsimd.dma_start(b2t_sb[:], b2t[:, :])
            b2cc_sb = sb.tile([DS, 1], F32)
            nc.sync.dma_start(b2cc_sb[:], b2cc[:, :])
            dbps_sb = sb.tile([T, DS], F16)
            nc.gpsimd.dma_start(dbps_sb[:], dbps[:, :])
            db1s_sb = sb.tile([T, HS], F16)
            nc.gpsimd.dma_start(db1s_sb[:], db1s[:, :])
            db2c_sb = sb.tile([T, DS], F16)
            nc.gpsimd.dma_start(db2c_sb[:], db2c[:, :])

            # delta slices: load fully into resident fp16 tiles so the DMA
            # streams from t=0 instead of waiting on the coefficients
            dwp_sb = sb.tile([128, 48 * DS], F16)
            nc.gpsimd.dma_start(
                dwp_sb[:].rearrange("p (tk m) -> p tk m", tk=48),
                dWps[:, :].rearrange("(tk p) m -> p tk m", tk=48, p=128))
            dw1_sb = sb.tile([128, 48 * HS], F16)
            nc.gpsimd.dma_start(
                dw1_sb[:].rearrange("p (tk m) -> p tk m", tk=48),
                dW1s[:, :].rearrange("(tk p) m -> p tk m", tk=48, p=128))
            dw2_sb = sb.tile([128, 24 * D], F16)
            nc.gpsimd.dma_start(
                dw2_sb[:].rearrange("p (tk m) -> p tk m", tk=24),
                dW2s[:, :].rearrange("(tk p) m -> p tk m", tk=24, p=128))

            # ---------- phase A: patch-mean pooling (fp32) ----------
            ra = sb.tile([126, 256], F32)
            rb = sb.tile([42, 256], F32)
            nc.vector.tensor_reduce(
                ra[:].rearrange("p (i j) -> p i j", i=16, j=16),
                xa[:].rearrange("p (i pj j) -> p i j pj", i=16, pj=14, j=16),
                op=ADD, axis=mybir.AxisListType.X)
            nc.vector.tensor_reduce(
                rb[:].rearrange("p (i j) -> p i j", i=16, j=16),
                xb[:].rearrange("p (i pj j) -> p i j pj", i=16, pj=14, j=16),
                op=ADD, axis=mybir.AxisListType.X)

            xloc = sb.tile([128, 6 * BL], F32)   # local xbar^T [ (c i j), bl ]
            for h in range(2):
                px = pst()[:, 0:12]
                nc.tensor.matmul(px, ra[:, 128 * h:128 * (h + 1)], selA_sb[:],
                                 start=True, stop=False)
                nc.tensor.matmul(px, rb[:, 128 * h:128 * (h + 1)], selB_sb[:],
                                 start=False, stop=True)
                for c in range(3):
                    kt = c * 2 + h
                    nc.scalar.copy(xloc[:, kt * BL:(kt + 1) * BL],
                                   px[:, c * BL:(c + 1) * BL])

            # Mask the local 4 batch columns into a full [768, 32] buffer and
            # AllReduce it: the summed result lands row-major so the re-land
            # is one contiguous DMA (vs a fragmented 16B-run gather from an
            # AllGather layout).
            bsel_sb = sb.tile([128, B], F32)
            nc.sync.dma_start(bsel_sb[:], bsel[:, :])
            xfull = sb.tile([128, 6 * B], F32)
            nc.vector.tensor_tensor(
                xfull[:].rearrange("p (kt r bl) -> p kt r bl", kt=6, r=8),
                xloc[:].rearrange("p (kt bl) -> p kt bl", kt=6)
                    .unsqueeze(2).broadcast_to([128, 6, 8, BL]),
                bsel_sb[:].unsqueeze(1).broadcast_to([128, 6, B])
                    .rearrange("p kt (r bl) -> p kt r bl", r=8),
                op=MULT)
            agx_in = dr.tile([D, B], F32)
            agx_out = dr.tile([D, B], F32)
            nc.sync.dma_start(
                agx_in[:].rearrange("(kt p) b -> p kt b", kt=6, p=128),
                xfull[:].rearrange("p (kt b) -> p kt b", kt=6))
            nc.gpsimd.collective_compute(
                "AllReduce", ADD, replica_groups=RG,
                ins=[agx_in[:].opt()], outs=[agx_out[:].opt()])
            xbar32 = sb.tile([128, 6 * B], F32)
            nc.sync.dma_start(
                xbar32[:].rearrange("p (kt b) -> p kt b", kt=6),
                agx_out[:].rearrange("(kt p) b -> p kt b", kt=6, p=128))
            xbar = sb.tile([128, 6 * B], F16)    # xbar^T [ (c i j), b ]
            nc.vector.tensor_copy(xbar[:], xbar32[:])
            xbar_v = xbar[:].rearrange("p (kt b) -> p kt b", kt=6)

            # ---------- phase B: base forward (H-sliced, fp16 matmuls) ------
            wp_v = wp_sb[:].rearrange("p (k m) -> p k m", k=6)
            F_sb = sb.tile([128, 6 * 32], F16)   # f^T
            for m in range(6):
                pf = pst()
                for k in range(6):
                    nc.tensor.matmul(pf[:], wp_v[:, k, 128 * m:128 * (m + 1)],
                                     xbar_v[:, k, :], start=(k == 0), stop=False)
                nc.tensor.matmul(pf[:], bpr_sb[0:1, 128 * m:128 * (m + 1)],
                                 ones_sb[0:1, :], start=False, stop=True)
                nc.scalar.copy(F_sb[:, m * 32:(m + 1) * 32], pf[:])
            F_v = F_sb[:].rearrange("p (k b) -> p k b", k=6)

            w1_v = w1_sb[:].rearrange("p (k m) -> p k m", k=6)
            a_sb = sb.tile([128, 3 * 32], F16)
            mask_sb = sb.tile([128, 3 * 32], F32)
            for m in range(3):
                pz = pst()
                for k in range(6):
                    nc.tensor.matmul(pz[:], w1_v[:, k, 128 * m:128 * (m + 1)],
                                     F_v[:, k, :], start=(k == 0), stop=False)
                nc.tensor.matmul(pz[:], b1r_sb[0:1, 128 * m:128 * (m + 1)],
                                 ones_sb[0:1, :], start=False, stop=True)
                nc.vector.tensor_scalar(a_sb[:, m * 32:(m + 1) * 32], pz[:],
                                        0.0, None, op0=mybir.AluOpType.max)
                nc.vector.tensor_scalar(mask_sb[:, m * 32:(m + 1) * 32], pz[:],
                                        0.0, None, op0=mybir.AluOpType.is_gt)
            a_v = a_sb[:].rearrange("p (k b) -> p k b", k=3)

            w2_v = w2_sb[:].rearrange("p (k m) -> p k m", k=3)
            basep_sb = sb.tile([128, 6 * 32], F16)   # partial base^T (no b2)
            for m in range(6):
                pb = pst()
                for k in range(3):
                    nc.tensor.matmul(pb[:], w2_v[:, k, 128 * m:128 * (m + 1)],
                                     a_v[:, k, :], start=(k == 0), stop=(k == 2))
                nc.scalar.copy(basep_sb[:, m * 32:(m + 1) * 32], pb[:])
            basep_v = basep_sb[:].rearrange("p (k b) -> p k b", k=6)

            # metanet pre-activation partial: mW1^T @ basep  [192, 32]
            mw1_v = mw1_sb[:].rearrange("p (k m) -> p k m", k=6)
            m1p0 = sb.tile([128, 32], F32)
            m1p1 = sb.tile([64, 32], F32)
            for mi, (mp, msl) in enumerate(((m1p0, slice(0, 128)),
                                            (m1p1, slice(128, 192)))):
                pm = pst(128 if mi == 0 else 64)
                for k in range(6):
                    nc.tensor.matmul(pm[:], mw1_v[:, k, msl], basep_v[:, k, :],
                                     start=(k == 0), stop=(k == 5))
                nc.scalar.copy(mp[:], pm[:])

            # metanet constant: mW1^T @ b2 + mb1  [192, 1]
            mc0 = sb.tile([128, 1], F32)
            mc1 = sb.tile([64, 1], F32)
            for mi, (mp, msl) in enumerate(((mc0, slice(0, 128)),
                                            (mc1, slice(128, 192)))):
                pm = ps.tile([128 if mi == 0 else 64, 1], F32, tag="ps",
                             name="pmc")
                for k in range(6):
                    nc.tensor.matmul(pm[:], mw1_v[:, k, msl], b2t_sb[:, k:k + 1],
                                     start=(k == 0), stop=False)
                nc.tensor.matmul(pm[:], mb1r_sb[0:1, msl], ones_sb[0:1, 0:1],
                                 start=False, stop=True)
                nc.scalar.copy(mp[:], pm[:])

            arm_in = dr.tile([MH, 32], F32)
            arm_out = dr.tile([MH, 32], F32)
            nc.sync.dma_start(arm_in[0:128, :], m1p0[:])
            nc.sync.dma_start(arm_in[128:192, :], m1p1[:])
            nc.gpsimd.collective_compute(
                "AllReduce", ADD, replica_groups=RG,
                ins=[arm_in[:].opt()], outs=[arm_out[:].opt()])
            m1s0 = sb.tile([128, 32], F32)
            m1s1 = sb.tile([64, 32], F32)
            nc.sync.dma_start(m1s0[:], arm_out[0:128, :])
            nc.sync.dma_start(m1s1[:], arm_out[128:192, :])
            m1a = sb.tile([128, 32], F16)
            m1b = sb.tile([64, 32], F16)
            nc.vector.tensor_scalar(m1a[:], m1s0[:], mc0[:], 0.0,
                                    op0=ADD, op1=mybir.AluOpType.max)
            nc.vector.tensor_scalar(m1b[:], m1s1[:], mc1[:], 0.0,
                                    op0=ADD, op1=mybir.AluOpType.max)

            # coefs cT' [48, 32], rows = p-block (order _PORDER) * 8 + t
            pc = pst(48)
            nc.tensor.matmul(pc[:], mw2_sb[:, 0:48], m1a[:],
                             start=True, stop=False)
            nc.tensor.matmul(pc[:], mw2_sb[0:64, 48:96], m1b[:],
                             start=False, stop=False)
            nc.tensor.matmul(pc[:], mb2p_sb[0:1, :], ones_sb[0:1, :],
                             start=False, stop=True)
            cT = sb.tile([48, 32], F32)
            nc.scalar.copy(cT[:], pc[:])

            # replicate scale rows (first 24) across 128 partitions via DRAM
            cdram = dr.tile([48, 32], F32)
            nc.sync.dma_start(cdram[:], cT[:])
            crep = sb.tile([128, 24 * 32], F32)
            nc.sync.dma_start(
                crep[:].rearrange("p (r b) -> p r b", r=24),
                cdram[0:24, :].unsqueeze(0).partition_broadcast(128))
            crep_v = crep[:].rearrange("p (pb t b) -> p pb t b", pb=3, t=8)
            # bias coefficient rows, re-landed at partition 0 for matmul rhs
            cb1f = sb.tile([T, 32], F32)
            cb3f = sb.tile([T, 32], F32)
            cb5f = sb.tile([T, 32], F32)
            nc.sync.dma_start(cb1f[:], cdram[24:32, :])
            nc.sync.dma_start(cb3f[:], cdram[32:40, :])
            nc.sync.dma_start(cb5f[:], cdram[40:48, :])
            cb1 = sb.tile([T, 32], F16)
            cb3 = sb.tile([T, 32], F16)
            cb5 = sb.tile([T, 32], F16)
            nc.vector.tensor_copy(cb1[:], cb1f[:])
            nc.vector.tensor_copy(cb3[:], cb3f[:])
            nc.vector.tensor_copy(cb5[:], cb5f[:])

            # ---------- phase C: per-task scaled rhs copies (fp16) ----------
            xts = sb.tile([128, T * 6 * 32], F16)
            nc.vector.tensor_tensor(
                xts[:].rearrange("p (t k b) -> p t k b", t=T, k=6),
                xbar_v.unsqueeze(1).broadcast_to([128, T, 6, 32]),
                crep_v[:, 0].unsqueeze(2).broadcast_to([128, T, 6, 32]),
                op=MULT)
            xts_v = xts[:].rearrange("p (t k b) -> p t k b", t=T, k=6)

            fts = sb.tile([128, T * 6 * 32], F16)
            nc.vector.tensor_tensor(
                fts[:].rearrange("p (t k b) -> p t k b", t=T, k=6),
                F_v.unsqueeze(1).broadcast_to([128, T, 6, 32]),
                crep_v[:, 1].unsqueeze(2).broadcast_to([128, T, 6, 32]),
                op=MULT)
            fts_v = fts[:].rearrange("p (t k b) -> p t k b", t=T, k=6)

            ats = sb.tile([128, T * 3 * 32], F16)
            nc.vector.tensor_tensor(
                ats[:].rearrange("p (t k b) -> p t k b", t=T, k=3),
                a_v.unsqueeze(1).broadcast_to([128, T, 3, 32]),
                crep_v[:, 2].unsqueeze(2).broadcast_to([128, T, 3, 32]),
                op=MULT)
            ats_v = ats[:].rearrange("p (t k b) -> p t k b", t=T, k=3)

            # ---------- phase D: delta matmuls (fp16) ----------
            # df chunk [96, 32]
            dwp_v = dwp_sb[:].rearrange("p (tk m) -> p tk m", tk=48)
            pdf = pst(DS)
            for t in range(T):
                for k in range(6):
                    nc.tensor.matmul(pdf[:], dwp_v[:, t * 6 + k, :],
                                     xts_v[:, t, k, :],
                                     start=(t == 0 and k == 0), stop=False)
            nc.tensor.matmul(pdf[:], dbps_sb[:], cb1[:],
                             start=False, stop=True)
            df_sb = sb.tile([DS, 32], F32)
            nc.scalar.copy(df_sb[:], pdf[:])

            agd_in = dr.tile([DS, 32], F32)
            agd_out = dr.tile([D, 32], F32)
            nc.sync.dma_start(agd_in[:], df_sb[:])
            nc.gpsimd.collective_compute(
                "AllGather", BYP, replica_groups=RG,
                ins=[agd_in[:].opt()], outs=[agd_out[:].opt()])
            dfT32 = sb.tile([128, 6 * 32], F32)
            nc.sync.dma_start(
                dfT32[:].rearrange("p (k b) -> p k b", k=6),
                agd_out[:, :].rearrange("(k p) b -> p k b", k=6, p=128))
            dfT = sb.tile([128, 6 * 32], F16)
            nc.vector.tensor_copy(dfT[:], dfT32[:])
            dfT_v = dfT[:].rearrange("p (k b) -> p k b", k=6)

            # S_Q slice [384, 32]: sum_t dW1[t][:, hs]^T @ (c2-scaled f^T)
            dw1_v = dw1_sb[:].rearrange("p (tk m) -> p tk m", tk=48)
            psQ = [pst() for _ in range(3)]
            for tk in range(48):
                t, k = tk // 6, tk % 6
                for m in range(3):
                    nc.tensor.matmul(psQ[m][:],
                                     dw1_v[:, tk, 128 * m:128 * (m + 1)],
                                     fts_v[:, t, k, :],
                                     start=(tk == 0), stop=False)
            sq_sb = sb.tile([128, 3 * 32], F32)
            for m in range(3):
                nc.tensor.matmul(psQ[m][:], db1s_sb[:, 128 * m:128 * (m + 1)],
                                 cb3[:], start=False, stop=True)
                nc.scalar.copy(sq_sb[:, m * 32:(m + 1) * 32], psQ[m][:])
            sq_v = sq_sb[:].rearrange("p (k b) -> p k b", k=3)

            # R partial [768, 32]: sum_t dW2[t][hs, :]^T @ (c4-scaled a^T)
            dw2_v = dw2_sb[:].rearrange("p (tk m) -> p tk m", tk=24)
            psR = [pst() for _ in range(6)]
            for tk in range(24):
                t, k = tk // 3, tk % 3
                for m in range(6):
                    nc.tensor.matmul(psR[m][:],
                                     dw2_v[:, tk, 128 * m:128 * (m + 1)],
                                     ats_v[:, t, k, :],
                                     start=(tk == 0), stop=(tk == 23))
            R_sb = sb.tile([128, 6 * 32], F32)
            for m in range(6):
                nc.scalar.copy(R_sb[:, m * 32:(m + 1) * 32], psR[m][:])
            R_v = R_sb[:].rearrange("p (k b) -> p k b", k=6)

            # ---------- phase E: tail ----------
            da_sb = sb.tile([128, 3 * 32], F16)
            tmp_sb = sb.tile([128, 3 * 32], F32)
            for m in range(3):
                pz = pst()
                for k in range(6):
                    nc.tensor.matmul(pz[:], w1_v[:, k, 128 * m:128 * (m + 1)],
                                     dfT_v[:, k, :], start=(k == 0),
                                     stop=(k == 5))
                nc.vector.tensor_tensor(tmp_sb[:, m * 32:(m + 1) * 32], pz[:],
                                        sq_v[:, m, :], op=ADD)
                nc.vector.tensor_tensor(da_sb[:, m * 32:(m + 1) * 32],
                                        tmp_sb[:, m * 32:(m + 1) * 32],
                                        mask_sb[:, m * 32:(m + 1) * 32],
                                        op=MULT)
            da_v = da_sb[:].rearrange("p (k b) -> p k b", k=3)

            contrib = sb.tile([128, 6 * 32], F32)
            for m in range(6):
                po = pst()
                for k in range(3):
                    nc.tensor.matmul(po[:], w2_v[:, k, 128 * m:128 * (m + 1)],
                                     da_v[:, k, :], start=(k == 0),
                                     stop=(k == 2))
                nc.vector.tensor_tensor(tmp_sb[:, 0:32], po[:],
                                        R_v[:, m, :], op=ADD)
                nc.vector.tensor_tensor(contrib[:, m * 32:(m + 1) * 32],
                                        tmp_sb[:, 0:32],
                                        basep_v[:, m, :], op=ADD)

            # db2 bias term (local, added post-ReduceScatter)
            pb2 = pst(DS)
            nc.tensor.matmul(pb2[:], db2c_sb[:], cb5[:],
                             start=True, stop=True)
            b2term = sb.tile([DS, 32], F32)
            nc.scalar.copy(b2term[:], pb2[:])

            rs_in = dr.tile([D, 32], F32)
            rs_out = dr.tile([DS, 32], F32)
            nc.sync.dma_start(
                rs_in[:].rearrange("(k p) b -> p k b", k=6, p=128),
                contrib[:].rearrange("p (k b) -> p k b", k=6))
            nc.gpsimd.collective_compute(
                "ReduceScatter", ADD, replica_groups=RG,
                ins=[rs_in[:].opt()], outs=[rs_out[:].opt()])
            fin = sb.tile([DS, 32], F32)
            nc.sync.dma_start(fin[:], rs_out[:, :])
            fin2 = sb.tile([DS, 32], F32)
            nc.vector.tensor_tensor(fin2[:], fin[:], b2term[:], op=ADD)
            out_sb = sb.tile([DS, 32], F32)
            nc.vector.tensor_scalar(out_sb[:], fin2[:], b2cc_sb[:], None,
                                    op0=ADD)
            nc.sync.dma_start(out[:, :], out_sb[:])

    nc.compile()
    return nc


_NC_CACHE = None


def _get_nc():
    global _NC_CACHE
    if _NC_CACHE is None:
        _NC_CACHE = _build_nc()
    return _NC_CACHE


_RUN_CACHE = None


def _get_runner():
    """Mirror of bass2jax.run_bass_via_pjrt's multi-core path, but inputs are
    device_put + block_until_ready'ed BEFORE the execute call so all 8 cores
    start with data resident (minimizes the NEFF-start skew barrier)."""
    global _RUN_CACHE
    if _RUN_CACHE is not None:
        return _RUN_CACHE
    import jax
    from jax.sharding import Mesh, PartitionSpec, NamedSharding
    from jax.experimental.shard_map import shard_map
    from concourse import bass2jax, mybir as _mybir

    nc = _get_nc()
    bass2jax.install_neuronx_cc_hook()

    in_names, out_names, out_avals, zero_shapes = [], [], [], []
    partition_name = (nc.partition_id_tensor.name
                      if nc.partition_id_tensor else None)
    for alloc in nc.m.functions[0].allocations:
        if not isinstance(alloc, _mybir.MemoryLocationSet):
            continue
        name = alloc.memorylocations[0].name
        if alloc.kind == "ExternalInput":
            if name != partition_name:
                in_names.append(name)
        elif alloc.kind == "ExternalOutput":
            shape = tuple(alloc.tensor_shape)
            dtype = _mybir.dt.np(alloc.dtype)
            out_names.append(name)
            out_avals.append(jax.core.ShapedArray(shape, dtype))
            zero_shapes.append((shape, dtype))
    n_params = len(in_names)
    n_outs = len(out_avals)
    all_in_names = list(in_names) + list(out_names)
    if partition_name is not None:
        all_in_names.append(partition_name)

    def _body(*args):
        operands = list(args)
        if partition_name is not None:
            operands.append(bass2jax.partition_id_tensor())
        outs = bass2jax._bass_exec_p.bind(
            *operands,
            out_avals=tuple(out_avals),
            in_names=tuple(all_in_names),
            out_names=tuple(out_names),
            lowering_input_output_aliases=(),
            sim_require_finite=True,
            sim_require_nnan=True,
            nc=nc,
        )
        return tuple(outs)

    devices = jax.devices()[:NCORES]
    mesh = Mesh(np.asarray(devices), ("core",))
    in_specs = (PartitionSpec("core"),) * (n_params + n_outs)
    out_specs = (PartitionSpec("core"),) * len(out_names)
    donate = tuple(range(n_params, n_params + n_outs))
    sharded = jax.jit(
        shard_map(_body, mesh=mesh, in_specs=in_specs, out_specs=out_specs,
                  check_rep=False),
        donate_argnums=donate, keep_unused=True)
    sh = NamedSharding(mesh, PartitionSpec("core"))

    def run(in_maps):
        per_core = [[np.asarray(m[name]) for name in in_names]
                    for m in in_maps]
        concat_in = [
            jax.device_put(
                np.concatenate([per_core[c][i] for c in range(NCORES)],
                               axis=0), sh)
            for i in range(n_params)]
        concat_zeros = [
            jax.device_put(
                np.zeros((NCORES * s[0], *s[1:]), dt), sh)
            for (s, dt) in zero_shapes]
        jax.block_until_ready(concat_in)
        jax.block_until_ready(concat_zeros)
        out_arrs = sharded(*concat_in, *concat_zeros)
        out_arrs = jax.block_until_ready(out_arrs)
        return [
            {name: np.asarray(out_arrs[i]).reshape(
                NCORES, *out_avals[i].shape)[c]
             for i, name in enumerate(out_names)}
            for c in range(NCORES)
        ]

    _RUN_CACHE = run
    return run


def _make_in_maps(x, Wp, bp, W1, b1, W2, b2,
                  dWp, dbp, dW1, db1, dW2, db2,
                  mW1, mb1, mW2, mb2):
    x = np.asarray(x, dtype=np.float32)
    f32 = lambda a: np.ascontiguousarray(np.asarray(a), dtype=np.float32)
    Wp, bp, W1, b1, W2, b2 = map(f32, (Wp, bp, W1, b1, W2, b2))
    dWp, dbp, dW1, db1, dW2, db2 = map(f32, (dWp, dbp, dW1, db1, dW2, db2))
    mW1, mb1, mW2, mb2 = map(f32, (mW1, mb1, mW2, mb2))

    perm = _metanet_perm()
    mW2p = np.ascontiguousarray(mW2[:, perm])
    mb2p = np.ascontiguousarray(mb2[perm])[None, :]

    selA = np.zeros((126, 12), dtype=np.float32)
    for b in range(3):
        for c in range(3):
            for pi in range(14):
                selA[b * 42 + c * 14 + pi, c * 4 + b] = 1.0 / NP
    selB = np.zeros((42, 12), dtype=np.float32)
    for c in range(3):
        for pi in range(14):
            selB[c * 14 + pi, c * 4 + 3] = 1.0 / NP

    ones = np.ones((1, 32), dtype=np.float32)
    b2t = np.ascontiguousarray(b2.reshape(6, 128).T)
    bsel_rows = []
    for i in range(NCORES):
        r = np.zeros((128, B), dtype=np.float32)
        r[:, BL * i:BL * (i + 1)] = 1.0
        bsel_rows.append(r)

    in_maps = []
    for i in range(NCORES):
        hs = slice(HS * i, HS * (i + 1))
        dsl = slice(DS * i, DS * (i + 1))
        m = {
            "xs": np.ascontiguousarray(x[BL * i:BL * (i + 1)]).reshape(168, 3584),
            "selA": selA, "selB": selB, "ones": ones,
            "Wp": Wp, "bpr": bp[None, :],
            "W1s": np.ascontiguousarray(W1[:, hs]), "b1r": b1[None, hs],
            "W2s": np.ascontiguousarray(W2[hs, :]),
            "mW1": mW1, "mb1r": mb1[None, :],
            "mW2p": mW2p, "mb2p": mb2p,
            "b2t": b2t, "b2cc": b2[dsl, None],
            "dWps": np.ascontiguousarray(dWp[:, :, dsl]).reshape(T * D, DS),
            "dW1s": np.ascontiguousarray(dW1[:, :, hs]).reshape(T * D, HS),
            "dW2s": np.ascontiguousarray(dW2[:, hs, :]).reshape(T * HS, D),
            "dbps": np.ascontiguousarray(dbp[:, dsl]),
            "db1s": np.ascontiguousarray(db1[:, hs]),
            "db2c": np.ascontiguousarray(db2[:, dsl]),
            "bsel": bsel_rows[i],
        }
        in_maps.append(m)
    return in_maps


def _assemble(results):
    chunks = [results[i]["out"] for i in range(NCORES)]
    full = np.concatenate(chunks, axis=0)      # [768, 32]
    return np.ascontiguousarray(full.T).astype(np.float32)   # [32, 768]


def kernel(**inputs) -> np.ndarray:
    in_maps = _make_in_maps(**inputs)
    results = _get_runner()(in_maps)
    return _assemble(results)


def kernel_traced(**inputs):
    """Like kernel() but returns (output, exec_time_ns) via neuron-profile.

    Uses the same pre-staged runner as kernel(); wraps the execute call in
    the axon NTFF profiling hook (registered by the caller / test harness).
    """
    import tempfile
    from antenv.axon_hooks import get_axon_ntff_profile_hook
    import gauge.profiler
    from concourse._compat import FishPath
    from concourse.bass_utils import _process_ntff_profile

    in_maps = _make_in_maps(**inputs)
    run = _get_runner()
    # warm-up execution (compiles + caches the executable)
    run(in_maps)

    hook = get_axon_ntff_profile_hook()
    neff_dir = tempfile.mkdtemp()
    with hook(neff_dir, list(range(NCORES))):
        results = run(in_maps)

    profile = gauge.profiler.Profile(
        profile_path=FishPath(neff_dir),
        kernel_dev_mode=True, profile_on_exit=False,
        bass_kernel=_get_nc().m, offline_processing=True,
        fname="*_body*", metadata={})
    pr = _process_ntff_profile(profile, neff_dir, _get_nc(),
                               list(range(NCORES)), list(range(NCORES)),
                               False, {}, trace_events=False)
    return _assemble(results), pr.exec_time_ns


# revision 28
# speedup vs baseline: 1.0338x; 1.0338x over previous
"""Trainium2 Bass kernel for nn_MetaNetLinearizedModel (8-core SPMD).

Math: func0 takes the patch-mean immediately after the first affine map, so
the whole per-patch computation collapses to the patch-mean vector xbar:
    f  = xbar @ Wp + bp          (xbar = patches.mean(axis=0))
    z1 = f @ W1 + b1 ; a = relu(z1) ; base = a @ W2 + b2
    coefs c[b,t,p] from MetaNet(base)
JVP term (per sample b), using linearity of the task-vector sums:
    df  = sum_t c0 * (xbar @ dWp[t]) + sum_t c1 * dbp[t]
    dz1 = df @ W1 + sum_t c2 * (f @ dW1[t]) + sum_t c3 * db1[t]
    da  = (z1 > 0) * dz1
    out = base + da @ W2 + sum_t c4 * (a @ dW2[t]) + sum_t c5 * db2[t]

Sharding (core i of 8):
  - batch slice 4i:4i+4 of x for the patch-mean (AllGather -> full xbar)
  - H-slice 384i:384(i+1) of W1/W2 for base fwd + tail (partials AllReduced /
    ReduceScattered)
  - task contraction slices of the delta tensors: dW1[:, :, Hslice],
    dW2[:, Hslice, :], dWp[:, :, Dchunk] so each core reads 1/8 of the
    deltas; the per-(b,t) coefficient scaling is folded into 8 scaled copies
    of the rhs activations and the task sum K-accumulates in PSUM.
Everything computed in transposed layout: features on partitions, batch (32)
on the free dim, so weights act as the stationary matmul operand in their
native [K, M] layout.  Matmul operands are fp16 (cast in-flight by gpsimd
DMAs); accumulation is fp32 in PSUM; the patch-mean pooling is fp32.
"""

import numpy as np

import concourse.bacc as bacc
import concourse.mybir as mybir
import concourse.tile as tile
from concourse.bass_utils import run_bass_kernel_spmd

F32 = mybir.dt.float32
F16 = mybir.dt.float16

NCORES = 8
B = 32          # batch
BL = B // NCORES  # local batch = 4
D = 768
H = 3072
T = 8
MH = 192        # metanet hidden
HS = H // NCORES   # 384 H-slice
DS = D // NCORES   # 96  D-chunk
NP = 196        # patches

# permutation of metanet output columns: p-major, even p blocks first so the
# scale rows (p in {0,2,4}) are contiguous, then the bias rows (p in {1,3,5}).
_PORDER = [0, 2, 4, 1, 3, 5]


def _metanet_perm():
    cols = []
    for p in _PORDER:
        for t in range(T):
            cols.append(t * 6 + p)
    return np.array(cols, dtype=np.int64)


def _build_nc():
    nc = bacc.Bacc("TRN2", target_bir_lowering=False, debug=False,
                   num_devices=NCORES)

    def inp(name, shape):
        return nc.dram_tensor(name, list(shape), F32, kind="ExternalInput")

    xs = inp("xs", [168, 3584])        # local 4 samples, [ (b c pi), (i pj j) ]
    selA = inp("selA", [126, 12])
    selB = inp("selB", [42, 12])
    ones = inp("ones", [1, 32])
    Wp = inp("Wp", [D, D])
    bpr = inp("bpr", [1, D])
    W1s = inp("W1s", [D, HS])
    b1r = inp("b1r", [1, HS])
    W2s = inp("W2s", [HS, D])
    mW1 = inp("mW1", [D, MH])
    mb1r = inp("mb1r", [1, MH])
    mW2p = inp("mW2p", [MH, 48])
    mb2p = inp("mb2p", [1, 48])
    b2t = inp("b2t", [128, 6])         # b2 as [128, 6] (col = k-tile)
    b2cc = inp("b2cc", [DS, 1])        # b2 chunk, per-partition scalar
    dWps = inp("dWps", [T * D, DS])    # dWp[:, :, dchunk]
    dW1s = inp("dW1s", [T * D, HS])    # dW1[:, :, hslice]
    dW2s = inp("dW2s", [T * HS, D])    # dW2[:, hslice, :]
    dbps = inp("dbps", [T, DS])
    db1s = inp("db1s", [T, HS])
    db2c = inp("db2c", [T, DS])
    bsel = inp("bsel", [128, B])       # 1.0 at this core's batch columns

    out = nc.dram_tensor("out", [DS, B], F32, kind="ExternalOutput")

    RG = [list(range(NCORES))]
    ADD = mybir.AluOpType.add
    BYP = mybir.AluOpType.bypass
    MULT = mybir.AluOpType.mult

    with tile.TileContext(nc) as tc:
        with tc.tile_pool(name="sb", bufs=1) as sb, \
             tc.tile_pool(name="ps", bufs=8, space="PSUM") as ps, \
             tc.tile_pool(name="dram", bufs=1, space="DRAM") as dr:

            def pst(p=128):
                return ps.tile([p, 32], F32, tag="ps", name="pst")

            # ---------- small/param DMAs (phase 1 needs) ----------
            selA_sb = sb.tile([126, 12], F32)
            selB_sb = sb.tile([42, 12], F32)
            ones_sb = sb.tile([1, 32], F16)
            nc.sync.dma_start(selA_sb[:], selA[:, :])
            nc.sync.dma_start(selB_sb[:], selB[:, :])
            nc.gpsimd.dma_start(ones_sb[:], ones[:, :])

            xa = sb.tile([126, 3584], F32)
            xb = sb.tile([42, 3584], F32)
            xa_dma = nc.sync.dma_start(xa[:], xs[0:126, :])
            xb_dma = nc.sync.dma_start(xb[:], xs[126:168, :])

            wp_sb = sb.tile([128, 6 * D], F16)
            wp_dma = nc.gpsimd.dma_start(
                wp_sb[:].rearrange("p (k m) -> p k m", k=6),
                Wp[:, :].rearrange("(k p) m -> p k m", k=6, p=128))
            # Gate the (FIFO) gpsimd prefetch stream behind the x tiles so the
            # pooling input doesn't contend with 35MB of weight prefetch.
            tile.add_dep_helper(wp_dma.ins, xa_dma.ins, sync=True,
                                reason="x before weight prefetch")
            tile.add_dep_helper(wp_dma.ins, xb_dma.ins, sync=True,
                                reason="x before weight prefetch")
            bpr_sb = sb.tile([1, D], F16)
            nc.gpsimd.dma_start(bpr_sb[:], bpr[:, :])

            w1_sb = sb.tile([128, 6 * HS], F16)
            nc.gpsimd.dma_start(
                w1_sb[:].rearrange("p (k m) -> p k m", k=6),
                W1s[:, :].rearrange("(k p) m -> p k m", k=6, p=128))
            b1r_sb = sb.tile([1, HS], F16)
            nc.gpsimd.dma_start(b1r_sb[:], b1r[:, :])

            w2_sb = sb.tile([128, 3 * D], F16)
            nc.gpsimd.dma_start(
                w2_sb[:].rearrange("p (k m) -> p k m", k=3),
                W2s[:, :].rearrange("(k p) m -> p k m", k=3, p=128))

            mw1_sb = sb.tile([128, 6 * MH], F16)
            nc.gpsimd.dma_start(
                mw1_sb[:].rearrange("p (k m) -> p k m", k=6),
                mW1[:, :].rearrange("(k p) m -> p k m", k=6, p=128))
            mb1r_sb = sb.tile([1, MH], F16)
            nc.gpsimd.dma_start(mb1r_sb[:], mb1r[:, :])
            mw2_sb = sb.tile([128, 96], F16)
            nc.gpsimd.dma_start(mw2_sb[:, 0:48], mW2p[0:128, :])
            nc.gpsimd.dma_start(mw2_sb[0:64, 48:96], mW2p[128:192, :])
            mb2p_sb = sb.tile([1, 48], F16)
            nc.gpsimd.dma_start(mb2p_sb[:], mb2p[:, :])
            b2t_sb = sb.tile([128, 6], F16)
            nc.gpsimd.dma_start(b2t_sb[:], b2t[:, :])
            b2cc_sb = sb.tile([DS, 1], F32)
            nc.sync.dma_start(b2cc_sb[:], b2cc[:, :])
            dbps_sb = sb.tile([T, DS], F16)
            nc.gpsimd.dma_start(dbps_sb[:], dbps[:, :])
            db1s_sb = sb.tile([T, HS], F16)
            nc.gpsimd.dma_start(db1s_sb[:], db1s[:, :])
            db2c_sb = sb.tile([T, DS], F16)
            nc.gpsimd.dma_start(db2c_sb[:], db2c[:, :])

            # delta slices: load fully into resident fp16 tiles so the DMA
            # streams from t=0 instead of waiting on the coefficients
            dwp_sb = sb.tile([128, 48 * DS], F16)
            nc.gpsimd.dma_start(
                dwp_sb[:].rearrange("p (tk m) -> p tk m", tk=48),
                dWps[:, :].rearrange("(tk p) m -> p tk m", tk=48, p=128))
            dw1_sb = sb.tile([128, 48 * HS], F16)
            dw1_dma = nc.gpsimd.dma_start(
                dw1_sb[:].rearrange("p (tk m) -> p tk m", tk=48),
                dW1s[:, :].rearrange("(tk p) m -> p tk m", tk=48, p=128))
            dw2_sb = sb.tile([128, 24 * D], F16)
            nc.gpsimd.dma_start(
                dw2_sb[:].rearrange("p (tk m) -> p tk m", tk=24),
                dW2s[:, :].rearrange("(tk p) m -> p tk m", tk=24, p=128))

            # ---------- phase A: patch-mean pooling (fp32) ----------
            ra = sb.tile([126, 256], F32)
            rb = sb.tile([42, 256], F32)
            nc.vector.tensor_reduce(
                ra[:].rearrange("p (i j) -> p i j", i=16, j=16),
                xa[:].rearrange("p (i pj j) -> p i j pj", i=16, pj=14, j=16),
                op=ADD, axis=mybir.AxisListType.X)
            nc.vector.tensor_reduce(
                rb[:].rearrange("p (i j) -> p i j", i=16, j=16),
                xb[:].rearrange("p (i pj j) -> p i j pj", i=16, pj=14, j=16),
                op=ADD, axis=mybir.AxisListType.X)

            xloc = sb.tile([128, 6 * BL], F32)   # local xbar^T [ (c i j), bl ]
            for h in range(2):
                px = pst()[:, 0:12]
                nc.tensor.matmul(px, ra[:, 128 * h:128 * (h + 1)], selA_sb[:],
                                 start=True, stop=False)
                nc.tensor.matmul(px, rb[:, 128 * h:128 * (h + 1)], selB_sb[:],
                                 start=False, stop=True)
                for c in range(3):
                    kt = c * 2 + h
                    nc.scalar.copy(xloc[:, kt * BL:(kt + 1) * BL],
                                   px[:, c * BL:(c + 1) * BL])

            # Mask the local 4 batch columns into a full [768, 32] buffer and
            # AllReduce it: the summed result lands row-major so the re-land
            # is one contiguous DMA (vs a fragmented 16B-run gather from an
            # AllGather layout).
            bsel_sb = sb.tile([128, B], F32)
            nc.sync.dma_start(bsel_sb[:], bsel[:, :])
            xfull = sb.tile([128, 6 * B], F32)
            nc.vector.tensor_tensor(
                xfull[:].rearrange("p (kt r bl) -> p kt r bl", kt=6, r=8),
                xloc[:].rearrange("p (kt bl) -> p kt bl", kt=6)
                    .unsqueeze(2).broadcast_to([128, 6, 8, BL]),
                bsel_sb[:].unsqueeze(1).broadcast_to([128, 6, B])
                    .rearrange("p kt (r bl) -> p kt r bl", r=8),
                op=MULT)
            agx_in = dr.tile([D, B], F32)
            agx_out = dr.tile([D, B], F32)
            nc.sync.dma_start(
                agx_in[:].rearrange("(kt p) b -> p kt b", kt=6, p=128),
                xfull[:].rearrange("p (kt b) -> p kt b", kt=6))
            nc.gpsimd.collective_compute(
                "AllReduce", ADD, replica_groups=RG,
                ins=[agx_in[:].opt()], outs=[agx_out[:].opt()])
            xbar32 = sb.tile([128, 6 * B], F32)
            xbar_dma = nc.sync.dma_start(
                xbar32[:].rearrange("p (kt b) -> p kt b", kt=6),
                agx_out[:].rearrange("(kt p) b -> p kt b", kt=6, p=128))
            # Hold the 19MB dw1/dw2 prefetch until the latency-critical first
            # AllReduce + re-land are done — they need a quiet HBM, and the
            # deltas aren't consumed until well after the coefficients.
            tile.add_dep_helper(dw1_dma.ins, xbar_dma.ins, sync=True,
                                reason="delta prefetch after xbar allreduce")
            xbar = sb.tile([128, 6 * B], F16)    # xbar^T [ (c i j), b ]
            nc.vector.tensor_copy(xbar[:], xbar32[:])
            xbar_v = xbar[:].rearrange("p (kt b) -> p kt b", kt=6)

            # ---------- phase B: base forward (H-sliced, fp16 matmuls) ------
            wp_v = wp_sb[:].rearrange("p (k m) -> p k m", k=6)
            F_sb = sb.tile([128, 6 * 32], F16)   # f^T
            for m in range(6):
                pf = pst()
                for k in range(6):
                    nc.tensor.matmul(pf[:], wp_v[:, k, 128 * m:128 * (m + 1)],
                                     xbar_v[:, k, :], start=(k == 0), stop=False)
                nc.tensor.matmul(pf[:], bpr_sb[0:1, 128 * m:128 * (m + 1)],
                                 ones_sb[0:1, :], start=False, stop=True)
                nc.scalar.copy(F_sb[:, m * 32:(m + 1) * 32], pf[:])
            F_v = F_sb[:].rearrange("p (k b) -> p k b", k=6)

            w1_v = w1_sb[:].rearrange("p (k m) -> p k m", k=6)
            a_sb = sb.tile([128, 3 * 32], F16)
            mask_sb = sb.tile([128, 3 * 32], F32)
            for m in range(3):
                pz = pst()
                for k in range(6):
                    nc.tensor.matmul(pz[:], w1_v[:, k, 128 * m:128 * (m + 1)],
                                     F_v[:, k, :], start=(k == 0), stop=False)
                nc.tensor.matmul(pz[:], b1r_sb[0:1, 128 * m:128 * (m + 1)],
                                 ones_sb[0:1, :], start=False, stop=True)
                nc.vector.tensor_scalar(a_sb[:, m * 32:(m + 1) * 32], pz[:],
                                        0.0, None, op0=mybir.AluOpType.max)
                nc.vector.tensor_scalar(mask_sb[:, m * 32:(m + 1) * 32], pz[:],
                                        0.0, None, op0=mybir.AluOpType.is_gt)
            a_v = a_sb[:].rearrange("p (k b) -> p k b", k=3)

            w2_v = w2_sb[:].rearrange("p (k m) -> p k m", k=3)
            basep_sb = sb.tile([128, 6 * 32], F16)   # partial base^T (no b2)
            for m in range(6):
                pb = pst()
                for k in range(3):
                    nc.tensor.matmul(pb[:], w2_v[:, k, 128 * m:128 * (m + 1)],
                                     a_v[:, k, :], start=(k == 0), stop=(k == 2))
                nc.scalar.copy(basep_sb[:, m * 32:(m + 1) * 32], pb[:])
            basep_v = basep_sb[:].rearrange("p (k b) -> p k b", k=6)

            # metanet pre-activation partial: mW1^T @ basep  [192, 32]
            mw1_v = mw1_sb[:].rearrange("p (k m) -> p k m", k=6)
            m1p0 = sb.tile([128, 32], F32)
            m1p1 = sb.tile([64, 32], F32)
            for mi, (mp, msl) in enumerate(((m1p0, slice(0, 128)),
                                            (m1p1, slice(128, 192)))):
                pm = pst(128 if mi == 0 else 64)
                for k in range(6):
                    nc.tensor.matmul(pm[:], mw1_v[:, k, msl], basep_v[:, k, :],
                                     start=(k == 0), stop=(k == 5))
                nc.scalar.copy(mp[:], pm[:])

            # metanet constant: mW1^T @ b2 + mb1  [192, 1]
            mc0 = sb.tile([128, 1], F32)
            mc1 = sb.tile([64, 1], F32)
            for mi, (mp, msl) in enumerate(((mc0, slice(0, 128)),
                                            (mc1, slice(128, 192)))):
                pm = ps.tile([128 if mi == 0 else 64, 1], F32, tag="ps",
                             name="pmc")
                for k in range(6):
                    nc.tensor.matmul(pm[:], mw1_v[:, k, msl], b2t_sb[:, k:k + 1],
                                     start=(k == 0), stop=False)
                nc.tensor.matmul(pm[:], mb1r_sb[0:1, msl], ones_sb[0:1, 0:1],
                                 start=False, stop=True)
                nc.scalar.copy(mp[:], pm[:])

            arm_in = dr.tile([MH, 32], F32)
            arm_out = dr.tile([MH, 32], F32)
            nc.sync.dma_start(arm_in[0:128, :], m1p0[:])
            nc.sync.dma_start(arm_in[128:192, :], m1p1[:])
            nc.gpsimd.collective_compute(
                "AllReduce", ADD, replica_groups=RG,
                ins=[arm_in[:].opt()], outs=[arm_out[:].opt()])
            m1s0 = sb.tile([128, 32], F32)
            m1s1 = sb.tile([64, 32], F32)
            nc.sync.dma_start(m1s0[:], arm_out[0:128, :])
            nc.sync.dma_start(m1s1[:], arm_out[128:192, :])
            m1a = sb.tile([128, 32], F16)
            m1b = sb.tile([64, 32], F16)
            nc.vector.tensor_scalar(m1a[:], m1s0[:], mc0[:], 0.0,
                                    op0=ADD, op1=mybir.AluOpType.max)
            nc.vector.tensor_scalar(m1b[:], m1s1[:], mc1[:], 0.0,
                                    op0=ADD, op1=mybir.AluOpType.max)

            # coefs cT' [48, 32], rows = p-block (order _PORDER) * 8 + t
            pc = pst(48)
            nc.tensor.matmul(pc[:], mw2_sb[:, 0:48], m1a[:],
                             start=True, stop=False)
            nc.tensor.matmul(pc[:], mw2_sb[0:64, 48:96], m1b[:],
                             start=False, stop=False)
            nc.tensor.matmul(pc[:], mb2p_sb[0:1, :], ones_sb[0:1, :],
                             start=False, stop=True)
            cT = sb.tile([48, 32], F32)
            nc.scalar.copy(cT[:], pc[:])

            # replicate scale rows (first 24) across 128 partitions via DRAM
            cdram = dr.tile([48, 32], F32)
            nc.sync.dma_start(cdram[:], cT[:])
            crep = sb.tile([128, 24 * 32], F32)
            nc.sync.dma_start(
                crep[:].rearrange("p (r b) -> p r b", r=24),
                cdram[0:24, :].unsqueeze(0).partition_broadcast(128))
            crep_v = crep[:].rearrange("p (pb t b) -> p pb t b", pb=3, t=8)
            # bias coefficient rows, re-landed at partition 0 for matmul rhs
            cb1f = sb.tile([T, 32], F32)
            cb3f = sb.tile([T, 32], F32)
            cb5f = sb.tile([T, 32], F32)
            nc.sync.dma_start(cb1f[:], cdram[24:32, :])
            nc.sync.dma_start(cb3f[:], cdram[32:40, :])
            nc.sync.dma_start(cb5f[:], cdram[40:48, :])
            cb1 = sb.tile([T, 32], F16)
            cb3 = sb.tile([T, 32], F16)
            cb5 = sb.tile([T, 32], F16)
            nc.vector.tensor_copy(cb1[:], cb1f[:])
            nc.vector.tensor_copy(cb3[:], cb3f[:])
            nc.vector.tensor_copy(cb5[:], cb5f[:])

            # ---------- phase C: per-task scaled rhs copies (fp16) ----------
            xts = sb.tile([128, T * 6 * 32], F16)
            nc.vector.tensor_tensor(
                xts[:].rearrange("p (t k b) -> p t k b", t=T, k=6),
                xbar_v.unsqueeze(1).broadcast_to([128, T, 6, 32]),
                crep_v[:, 0].unsqueeze(2).broadcast_to([128, T, 6, 32]),
                op=MULT)
            xts_v = xts[:].rearrange("p (t k b) -> p t k b", t=T, k=6)

            fts = sb.tile([128, T * 6 * 32], F16)
            nc.vector.tensor_tensor(
                fts[:].rearrange("p (t k b) -> p t k b", t=T, k=6),
                F_v.unsqueeze(1).broadcast_to([128, T, 6, 32]),
                crep_v[:, 1].unsqueeze(2).broadcast_to([128, T, 6, 32]),
                op=MULT)
            fts_v = fts[:].rearrange("p (t k b) -> p t k b", t=T, k=6)

            ats = sb.tile([128, T * 3 * 32], F16)
            nc.vector.tensor_tensor(
                ats[:].rearrange("p (t k b) -> p t k b", t=T, k=3),
                a_v.unsqueeze(1).broadcast_to([128, T, 3, 32]),
                crep_v[:, 2].unsqueeze(2).broadcast_to([128, T, 3, 32]),
                op=MULT)
            ats_v = ats[:].rearrange("p (t k b) -> p t k b", t=T, k=3)

            # ---------- phase D: delta matmuls (fp16) ----------
            # df chunk [96, 32]
            dwp_v = dwp_sb[:].rearrange("p (tk m) -> p tk m", tk=48)
            pdf = pst(DS)
            for t in range(T):
                for k in range(6):
                    nc.tensor.matmul(pdf[:], dwp_v[:, t * 6 + k, :],
                                     xts_v[:, t, k, :],
                                     start=(t == 0 and k == 0), stop=False)
            nc.tensor.matmul(pdf[:], dbps_sb[:], cb1[:],
                             start=False, stop=True)
            df_sb = sb.tile([DS, 32], F32)
            nc.scalar.copy(df_sb[:], pdf[:])

            agd_in = dr.tile([DS, 32], F32)
            agd_out = dr.tile([D, 32], F32)
            nc.sync.dma_start(agd_in[:], df_sb[:])
            nc.gpsimd.collective_compute(
                "AllGather", BYP, replica_groups=RG,
                ins=[agd_in[:].opt()], outs=[agd_out[:].opt()])
            dfT32 = sb.tile([128, 6 * 32], F32)
            nc.sync.dma_start(
                dfT32[:].rearrange("p (k b) -> p k b", k=6),
                agd_out[:, :].rearrange("(k p) b -> p k b", k=6, p=128))
            dfT = sb.tile([128, 6 * 32], F16)
            nc.vector.tensor_copy(dfT[:], dfT32[:])
            dfT_v = dfT[:].rearrange("p (k b) -> p k b", k=6)

            # S_Q slice [384, 32]: sum_t dW1[t][:, hs]^T @ (c2-scaled f^T)
            dw1_v = dw1_sb[:].rearrange("p (tk m) -> p tk m", tk=48)
            psQ = [pst() for _ in range(3)]
            for tk in range(48):
                t, k = tk // 6, tk % 6
                for m in range(3):
                    nc.tensor.matmul(psQ[m][:],
                                     dw1_v[:, tk, 128 * m:128 * (m + 1)],
                                     fts_v[:, t, k, :],
                                     start=(tk == 0), stop=False)
            sq_sb = sb.tile([128, 3 * 32], F32)
            for m in range(3):
                nc.tensor.matmul(psQ[m][:], db1s_sb[:, 128 * m:128 * (m + 1)],
                                 cb3[:], start=False, stop=True)
                nc.scalar.copy(sq_sb[:, m * 32:(m + 1) * 32], psQ[m][:])
            sq_v = sq_sb[:].rearrange("p (k b) -> p k b", k=3)

            # R partial [768, 32]: sum_t dW2[t][hs, :]^T @ (c4-scaled a^T)
            dw2_v = dw2_sb[:].rearrange("p (tk m) -> p tk m", tk=24)
            psR = [pst() for _ in range(6)]
            for tk in range(24):
                t, k = tk // 3, tk % 3
                for m in range(6):
                    nc.tensor.matmul(psR[m][:],
                                     dw2_v[:, tk, 128 * m:128 * (m + 1)],
                                     ats_v[:, t, k, :],
                                     start=(tk == 0), stop=(tk == 23))
            R_sb = sb.tile([128, 6 * 32], F32)
            for m in range(6):
                nc.scalar.copy(R_sb[:, m * 32:(m + 1) * 32], psR[m][:])
            R_v = R_sb[:].rearrange("p (k b) -> p k b", k=6)

            # ---------- phase E: tail ----------
            da_sb = sb.tile([128, 3 * 32], F16)
            tmp_sb = sb.tile([128, 3 * 32], F32)
            for m in range(3):
                pz = pst()
                for k in range(6):
                    nc.tensor.matmul(pz[:], w1_v[:, k, 128 * m:128 * (m + 1)],
                                     dfT_v[:, k, :], start=(k == 0),
                                     stop=(k == 5))
                nc.vector.tensor_tensor(tmp_sb[:, m * 32:(m + 1) * 32], pz[:],
                                        sq_v[:, m, :], op=ADD)
                nc.vector.tensor_tensor(da_sb[:, m * 32:(m + 1) * 32],
                                        tmp_sb[:, m * 32:(m + 1) * 32],
                                        mask_sb[:, m * 32:(m + 1) * 32],
                                        op=MULT)
            da_v = da_sb[:].rearrange("p (k b) -> p k b", k=3)

            contrib = sb.tile([128, 6 * 32], F32)
            for m in range(6):
                po = pst()
                for k in range(3):
                    nc.tensor.matmul(po[:], w2_v[:, k, 128 * m:128 * (m + 1)],
                                     da_v[:, k, :], start=(k == 0),
                                     stop=(k == 2))
                nc.vector.tensor_tensor(tmp_sb[:, 0:32], po[:],
                                        R_v[:, m, :], op=ADD)
                nc.vector.tensor_tensor(contrib[:, m * 32:(m + 1) * 32],
                                        tmp_sb[:, 0:32],
                                        basep_v[:, m, :], op=ADD)

            # db2 bias term (local, added post-ReduceScatter)
            pb2 = pst(DS)
            nc.tensor.matmul(pb2[:], db2c_sb[:], cb5[:],
                             start=True, stop=True)
            b2term = sb.tile([DS, 32], F32)
            nc.vector.tensor_scalar(b2term[:], pb2[:], b2cc_sb[:], None,
                                    op0=ADD)

            rs_in = dr.tile([D, 32], F32)
            rs_out = dr.tile([DS, 32], F32)
            nc.sync.dma_start(
                rs_in[:].rearrange("(k p) b -> p k b", k=6, p=128),
                contrib[:].rearrange("p (k b) -> p k b", k=6))
            nc.gpsimd.collective_compute(
                "ReduceScatter", ADD, replica_groups=RG,
                ins=[rs_in[:].opt()], outs=[rs_out[:].opt()])
            fin = sb.tile([DS, 32], F32)
            nc.sync.dma_start(fin[:], rs_out[:, :])
            out_sb = sb.tile([DS, 32], F32)
            nc.vector.tensor_tensor(out_sb[:], fin[:], b2term[:], op=ADD)
            nc.sync.dma_start(out[:, :], out_sb[:])

    nc.compile()
    return nc


_NC_CACHE = None


def _get_nc():
    global _NC_CACHE
    if _NC_CACHE is None:
        _NC_CACHE = _build_nc()
    return _NC_CACHE


_RUN_CACHE = None


def _get_runner():
    """Mirror of bass2jax.run_bass_via_pjrt's multi-core path, but inputs are
    device_put + block_until_ready'ed BEFORE the execute call so all 8 cores
    start with data resident (minimizes the NEFF-start skew barrier)."""
    global _RUN_CACHE
    if _RUN_CACHE is not None:
        return _RUN_CACHE
    import jax
    from jax.sharding import Mesh, PartitionSpec, NamedSharding
    from jax.experimental.shard_map import shard_map
    from concourse import bass2jax, mybir as _mybir

    nc = _get_nc()
    bass2jax.install_neuronx_cc_hook()

    in_names, out_names, out_avals, zero_shapes = [], [], [], []
    partition_name = (nc.partition_id_tensor.name
                      if nc.partition_id_tensor else None)
    for alloc in nc.m.functions[0].allocations:
        if not isinstance(alloc, _mybir.MemoryLocationSet):
            continue
        name = alloc.memorylocations[0].name
        if alloc.kind == "ExternalInput":
            if name != partition_name:
                in_names.append(name)
        elif alloc.kind == "ExternalOutput":
            shape = tuple(alloc.tensor_shape)
            dtype = _mybir.dt.np(alloc.dtype)
            out_names.append(name)
            out_avals.append(jax.core.ShapedArray(shape, dtype))
            zero_shapes.append((shape, dtype))
    n_params = len(in_names)
    n_outs = len(out_avals)
    all_in_names = list(in_names) + list(out_names)
    if partition_name is not None:
        all_in_names.append(partition_name)

    def _body(*args):
        operands = list(args)
        if partition_name is not None:
            operands.append(bass2jax.partition_id_tensor())
        outs = bass2jax._bass_exec_p.bind(
            *operands,
            out_avals=tuple(out_avals),
            in_names=tuple(all_in_names),
            out_names=tuple(out_names),
            lowering_input_output_aliases=(),
            sim_require_finite=True,
            sim_require_nnan=True,
            nc=nc,
        )
        return tuple(outs)

    devices = jax.devices()[:NCORES]
    mesh = Mesh(np.asarray(devices), ("core",))
    in_specs = (PartitionSpec("core"),) * (n_params + n_outs)
    out_specs = (PartitionSpec("core"),) * len(out_names)
    donate = tuple(range(n_params, n_params + n_outs))
    sharded = jax.jit(
        shard_map(_body, mesh=mesh, in_specs=in_specs, out_specs=out_specs,
                  check_rep=False),
        donate_argnums=donate, keep_unused=True)
    sh = NamedSharding(mesh, PartitionSpec("core"))

    def run(in_maps):
        per_core = [[np.asarray(m[name]) for name in in_names]
                    for m in in_maps]
        concat_in = [
            jax.device_put(
                np.concatenate([per_core[c][i] for c in range(NCORES)],
                               axis=0), sh)
            for i in range(n_params)]
        concat_zeros = [
            jax.device_put(
                np.zeros((NCORES * s[0], *s[1:]), dt), sh)
            for (s, dt) in zero_shapes]
        jax.block_until_ready(concat_in)
        jax.block_until_ready(concat_zeros)
        out_arrs = sharded(*concat_in, *concat_zeros)
        out_arrs = jax.block_until_ready(out_arrs)
        return [
            {name: np.asarray(out_arrs[i]).reshape(
                NCORES, *out_avals[i].shape)[c]
             for i, name in enumerate(out_names)}
            for c in range(NCORES)
        ]

    _RUN_CACHE = run
    return run


def _make_in_maps(x, Wp, bp, W1, b1, W2, b2,
                  dWp, dbp, dW1, db1, dW2, db2,
                  mW1, mb1, mW2, mb2):
    x = np.asarray(x, dtype=np.float32)
    f32 = lambda a: np.ascontiguousarray(np.asarray(a), dtype=np.float32)
    Wp, bp, W1, b1, W2, b2 = map(f32, (Wp, bp, W1, b1, W2, b2))
    dWp, dbp, dW1, db1, dW2, db2 = map(f32, (dWp, dbp, dW1, db1, dW2, db2))
    mW1, mb1, mW2, mb2 = map(f32, (mW1, mb1, mW2, mb2))

    perm = _metanet_perm()
    mW2p = np.ascontiguousarray(mW2[:, perm])
    mb2p = np.ascontiguousarray(mb2[perm])[None, :]

    selA = np.zeros((126, 12), dtype=np.float32)
    for b in range(3):
        for c in range(3):
            for pi in range(14):
                selA[b * 42 + c * 14 + pi, c * 4 + b] = 1.0 / NP
    selB = np.zeros((42, 12), dtype=np.float32)
    for c in range(3):
        for pi in range(14):
            selB[c * 14 + pi, c * 4 + 3] = 1.0 / NP

    ones = np.ones((1, 32), dtype=np.float32)
    b2t = np.ascontiguousarray(b2.reshape(6, 128).T)
    bsel_rows = []
    for i in range(NCORES):
        r = np.zeros((128, B), dtype=np.float32)
        r[:, BL * i:BL * (i + 1)] = 1.0
        bsel_rows.append(r)

    in_maps = []
    for i in range(NCORES):
        hs = slice(HS * i, HS * (i + 1))
        dsl = slice(DS * i, DS * (i + 1))
        m = {
            "xs": np.ascontiguousarray(x[BL * i:BL * (i + 1)]).reshape(168, 3584),
            "selA": selA, "selB": selB, "ones": ones,
            "Wp": Wp, "bpr": bp[None, :],
            "W1s": np.ascontiguousarray(W1[:, hs]), "b1r": b1[None, hs],
            "W2s": np.ascontiguousarray(W2[hs, :]),
            "mW1": mW1, "mb1r": mb1[None, :],
            "mW2p": mW2p, "mb2p": mb2p,
            "b2t": b2t, "b2cc": b2[dsl, None],
            "dWps": np.ascontiguousarray(dWp[:, :, dsl]).reshape(T * D, DS),
            "dW1s": np.ascontiguousarray(dW1[:, :, hs]).reshape(T * D, HS),
            "dW2s": np.ascontiguousarray(dW2[:, hs, :]).reshape(T * HS, D),
            "dbps": np.ascontiguousarray(dbp[:, dsl]),
            "db1s": np.ascontiguousarray(db1[:, hs]),
            "db2c": np.ascontiguousarray(db2[:, dsl]),
            "bsel": bsel_rows[i],
        }
        in_maps.append(m)
    return in_maps


def _assemble(results):
    chunks = [results[i]["out"] for i in range(NCORES)]
    full = np.concatenate(chunks, axis=0)      # [768, 32]
    return np.ascontiguousarray(full.T).astype(np.float32)   # [32, 768]


def kernel(**inputs) -> np.ndarray:
    in_maps = _make_in_maps(**inputs)
    results = _get_runner()(in_maps)
    return _assemble(results)


def kernel_traced(**inputs):
    """Like kernel() but returns (output, exec_time_ns) via neuron-profile.

    Uses the same pre-staged runner as kernel(); wraps the execute call in
    the axon NTFF profiling hook (registered by the caller / test harness).
    """
    import tempfile
    from antenv.axon_hooks import get_axon_ntff_profile_hook
    import gauge.profiler
    from concourse._compat import FishPath
    from concourse.bass_utils import _process_ntff_profile

    in_maps = _make_in_maps(**inputs)
    run = _get_runner()
    # warm-up execution (compiles + caches the executable)
    run(in_maps)

    hook = get_axon_ntff_profile_hook()
    neff_dir = tempfile.mkdtemp()
    with hook(neff_dir, list(range(NCORES))):
        results = run(in_maps)

    profile = gauge.profiler.Profile(
        profile_path=FishPath(neff_dir),
        kernel_dev_mode=True, profile_on_exit=False,
        bass_kernel=_get_nc().m, offline_processing=True,
        fname="*_body*", metadata={})
    pr = _process_ntff_profile(profile, neff_dir, _get_nc(),
                               list(range(NCORES)), list(range(NCORES)),
                               False, {}, trace_events=False)
    return _assemble(results), pr.exec_time_ns


# revision 29
# speedup vs baseline: 1.0674x; 1.0325x over previous
"""Trainium2 Bass kernel for nn_MetaNetLinearizedModel (8-core SPMD).

Math: func0 takes the patch-mean immediately after the first affine map, so
the whole per-patch computation collapses to the patch-mean vector xbar:
    f  = xbar @ Wp + bp          (xbar = patches.mean(axis=0))
    z1 = f @ W1 + b1 ; a = relu(z1) ; base = a @ W2 + b2
    coefs c[b,t,p] from MetaNet(base)
JVP term (per sample b), using linearity of the task-vector sums:
    df  = sum_t c0 * (xbar @ dWp[t]) + sum_t c1 * dbp[t]
    dz1 = df @ W1 + sum_t c2 * (f @ dW1[t]) + sum_t c3 * db1[t]
    da  = (z1 > 0) * dz1
    out = base + da @ W2 + sum_t c4 * (a @ dW2[t]) + sum_t c5 * db2[t]

Sharding (core i of 8):
  - batch slice 4i:4i+4 of x for the patch-mean (AllGather -> full xbar)
  - H-slice 384i:384(i+1) of W1/W2 for base fwd + tail (partials AllReduced /
    ReduceScattered)
  - task contraction slices of the delta tensors: dW1[:, :, Hslice],
    dW2[:, Hslice, :], dWp[:, :, Dchunk] so each core reads 1/8 of the
    deltas; the per-(b,t) coefficient scaling is folded into 8 scaled copies
    of the rhs activations and the task sum K-accumulates in PSUM.
Everything computed in transposed layout: features on partitions, batch (32)
on the free dim, so weights act as the stationary matmul operand in their
native [K, M] layout.  Matmul operands are fp16 (cast in-flight by gpsimd
DMAs); accumulation is fp32 in PSUM; the patch-mean pooling is fp32.
"""

import numpy as np

import concourse.bacc as bacc
import concourse.mybir as mybir
import concourse.tile as tile
from concourse.bass_utils import run_bass_kernel_spmd

F32 = mybir.dt.float32
F16 = mybir.dt.float16

NCORES = 8
B = 32          # batch
BL = B // NCORES  # local batch = 4
D = 768
H = 3072
T = 8
MH = 192        # metanet hidden
HS = H // NCORES   # 384 H-slice
DS = D // NCORES   # 96  D-chunk
NP = 196        # patches

# permutation of metanet output columns: p-major, even p blocks first so the
# scale rows (p in {0,2,4}) are contiguous, then the bias rows (p in {1,3,5}).
_PORDER = [0, 2, 4, 1, 3, 5]


def _metanet_perm():
    cols = []
    for p in _PORDER:
        for t in range(T):
            cols.append(t * 6 + p)
    return np.array(cols, dtype=np.int64)


def _build_nc():
    nc = bacc.Bacc("TRN2", target_bir_lowering=False, debug=False,
                   num_devices=NCORES)

    def inp(name, shape):
        return nc.dram_tensor(name, list(shape), F32, kind="ExternalInput")

    xs = inp("xs", [168, 3584])        # local 4 samples, [ (b c pi), (i pj j) ]
    selA = inp("selA", [126, 12])
    selB = inp("selB", [42, 12])
    ones = inp("ones", [1, 32])
    Wp = inp("Wp", [D, D])
    bpr = inp("bpr", [1, D])
    W1s = inp("W1s", [D, HS])
    b1r = inp("b1r", [1, HS])
    W2s = inp("W2s", [HS, D])
    mW1 = inp("mW1", [D, MH])
    mb1r = inp("mb1r", [1, MH])
    mW2p = inp("mW2p", [MH, 48])
    mb2p = inp("mb2p", [1, 48])
    b2t = inp("b2t", [128, 6])         # b2 as [128, 6] (col = k-tile)
    b2cc = inp("b2cc", [DS, 1])        # b2 chunk, per-partition scalar
    dWps = inp("dWps", [T * D, DS])    # dWp[:, :, dchunk]
    dW1s = inp("dW1s", [T * D, HS])    # dW1[:, :, hslice]
    dW2s = inp("dW2s", [T * HS, D])    # dW2[:, hslice, :]
    dbps = inp("dbps", [T, DS])
    db1s = inp("db1s", [T, HS])
    db2c = inp("db2c", [T, DS])
    bsel = inp("bsel", [128, B])       # 1.0 at this core's batch columns

    out = nc.dram_tensor("out", [DS, B], F32, kind="ExternalOutput")

    RG = [list(range(NCORES))]
    ADD = mybir.AluOpType.add
    BYP = mybir.AluOpType.bypass
    MULT = mybir.AluOpType.mult

    with tile.TileContext(nc) as tc:
        with tc.tile_pool(name="sb", bufs=1) as sb, \
             tc.tile_pool(name="ps", bufs=8, space="PSUM") as ps, \
             tc.tile_pool(name="dram", bufs=1, space="DRAM") as dr:

            def pst(p=128):
                return ps.tile([p, 32], F32, tag="ps", name="pst")

            # ---------- small/param DMAs (phase 1 needs) ----------
            # x tiles first on the gpsimd SWDGE ring (fp16 cast halves the
            # bytes and doubles the DVE reduce rate); the ring drains FIFO so
            # everything else queues behind them.
            xa = sb.tile([126, 3584], F16)
            xb = sb.tile([42, 3584], F16)
            nc.gpsimd.dma_start(xa[:], xs[0:126, :])
            nc.gpsimd.dma_start(xb[:], xs[126:168, :])

            selA_sb = sb.tile([126, 12], F32)
            selB_sb = sb.tile([42, 12], F32)
            ones_sb = sb.tile([1, 32], F16)
            nc.sync.dma_start(selA_sb[:], selA[:, :])
            nc.sync.dma_start(selB_sb[:], selB[:, :])
            nc.gpsimd.dma_start(ones_sb[:], ones[:, :])

            wp_sb = sb.tile([128, 6 * D], F16)
            nc.gpsimd.dma_start(
                wp_sb[:].rearrange("p (k m) -> p k m", k=6),
                Wp[:, :].rearrange("(k p) m -> p k m", k=6, p=128))
            bpr_sb = sb.tile([1, D], F16)
            nc.gpsimd.dma_start(bpr_sb[:], bpr[:, :])

            w1_sb = sb.tile([128, 6 * HS], F16)
            nc.gpsimd.dma_start(
                w1_sb[:].rearrange("p (k m) -> p k m", k=6),
                W1s[:, :].rearrange("(k p) m -> p k m", k=6, p=128))
            b1r_sb = sb.tile([1, HS], F16)
            nc.gpsimd.dma_start(b1r_sb[:], b1r[:, :])

            w2_sb = sb.tile([128, 3 * D], F16)
            nc.gpsimd.dma_start(
                w2_sb[:].rearrange("p (k m) -> p k m", k=3),
                W2s[:, :].rearrange("(k p) m -> p k m", k=3, p=128))

            mw1_sb = sb.tile([128, 6 * MH], F16)
            nc.gpsimd.dma_start(
                mw1_sb[:].rearrange("p (k m) -> p k m", k=6),
                mW1[:, :].rearrange("(k p) m -> p k m", k=6, p=128))
            mb1r_sb = sb.tile([1, MH], F16)
            nc.gpsimd.dma_start(mb1r_sb[:], mb1r[:, :])
            mw2_sb = sb.tile([128, 96], F16)
            nc.gpsimd.dma_start(mw2_sb[:, 0:48], mW2p[0:128, :])
            nc.gpsimd.dma_start(mw2_sb[0:64, 48:96], mW2p[128:192, :])
            mb2p_sb = sb.tile([1, 48], F16)
            nc.gpsimd.dma_start(mb2p_sb[:], mb2p[:, :])
            b2t_sb = sb.tile([128, 6], F16)
            nc.gpsimd.dma_start(b2t_sb[:], b2t[:, :])
            b2cc_sb = sb.tile([DS, 1], F32)
            nc.sync.dma_start(b2cc_sb[:], b2cc[:, :])
            dbps_sb = sb.tile([T, DS], F16)
            nc.gpsimd.dma_start(dbps_sb[:], dbps[:, :])
            db1s_sb = sb.tile([T, HS], F16)
            nc.gpsimd.dma_start(db1s_sb[:], db1s[:, :])
            db2c_sb = sb.tile([T, DS], F16)
            nc.gpsimd.dma_start(db2c_sb[:], db2c[:, :])

            # delta slices: load fully into resident fp16 tiles so the DMA
            # streams from t=0 instead of waiting on the coefficients
            dwp_sb = sb.tile([128, 48 * DS], F16)
            nc.gpsimd.dma_start(
                dwp_sb[:].rearrange("p (tk m) -> p tk m", tk=48),
                dWps[:, :].rearrange("(tk p) m -> p tk m", tk=48, p=128))
            dw1_sb = sb.tile([128, 48 * HS], F16)
            dw1_dma = nc.gpsimd.dma_start(
                dw1_sb[:].rearrange("p (tk m) -> p tk m", tk=48),
                dW1s[:, :].rearrange("(tk p) m -> p tk m", tk=48, p=128))
            dw2_sb = sb.tile([128, 24 * D], F16)
            nc.gpsimd.dma_start(
                dw2_sb[:].rearrange("p (tk m) -> p tk m", tk=24),
                dW2s[:, :].rearrange("(tk p) m -> p tk m", tk=24, p=128))

            # ---------- phase A: patch-mean pooling (fp32) ----------
            ra = sb.tile([126, 256], F32)
            rb = sb.tile([42, 256], F32)
            nc.vector.tensor_reduce(
                ra[:].rearrange("p (i j) -> p i j", i=16, j=16),
                xa[:].rearrange("p (i pj j) -> p i j pj", i=16, pj=14, j=16),
                op=ADD, axis=mybir.AxisListType.X)
            nc.vector.tensor_reduce(
                rb[:].rearrange("p (i j) -> p i j", i=16, j=16),
                xb[:].rearrange("p (i pj j) -> p i j pj", i=16, pj=14, j=16),
                op=ADD, axis=mybir.AxisListType.X)

            xloc = sb.tile([128, 6 * BL], F32)   # local xbar^T [ (c i j), bl ]
            for h in range(2):
                px = pst()[:, 0:12]
                nc.tensor.matmul(px, ra[:, 128 * h:128 * (h + 1)], selA_sb[:],
                                 start=True, stop=False)
                nc.tensor.matmul(px, rb[:, 128 * h:128 * (h + 1)], selB_sb[:],
                                 start=False, stop=True)
                for c in range(3):
                    kt = c * 2 + h
                    nc.scalar.copy(xloc[:, kt * BL:(kt + 1) * BL],
                                   px[:, c * BL:(c + 1) * BL])

            # Mask the local 4 batch columns into a full [768, 32] buffer and
            # AllReduce it: the summed result lands row-major so the re-land
            # is one contiguous DMA (vs a fragmented 16B-run gather from an
            # AllGather layout).
            bsel_sb = sb.tile([128, B], F32)
            nc.sync.dma_start(bsel_sb[:], bsel[:, :])
            xfull = sb.tile([128, 6 * B], F32)
            nc.vector.tensor_tensor(
                xfull[:].rearrange("p (kt r bl) -> p kt r bl", kt=6, r=8),
                xloc[:].rearrange("p (kt bl) -> p kt bl", kt=6)
                    .unsqueeze(2).broadcast_to([128, 6, 8, BL]),
                bsel_sb[:].unsqueeze(1).broadcast_to([128, 6, B])
                    .rearrange("p kt (r bl) -> p kt r bl", r=8),
                op=MULT)
            agx_in = dr.tile([D, B], F32)
            agx_out = dr.tile([D, B], F32)
            nc.sync.dma_start(
                agx_in[:].rearrange("(kt p) b -> p kt b", kt=6, p=128),
                xfull[:].rearrange("p (kt b) -> p kt b", kt=6))
            nc.gpsimd.collective_compute(
                "AllReduce", ADD, replica_groups=RG,
                ins=[agx_in[:].opt()], outs=[agx_out[:].opt()])
            xbar32 = sb.tile([128, 6 * B], F32)
            xbar_dma = nc.sync.dma_start(
                xbar32[:].rearrange("p (kt b) -> p kt b", kt=6),
                agx_out[:].rearrange("(kt p) b -> p kt b", kt=6, p=128))
            # Hold the 19MB dw1/dw2 prefetch until the latency-critical first
            # AllReduce + re-land are done — they need a quiet HBM, and the
            # deltas aren't consumed until well after the coefficients.
            tile.add_dep_helper(dw1_dma.ins, xbar_dma.ins, sync=True,
                                reason="delta prefetch after xbar allreduce")
            xbar = sb.tile([128, 6 * B], F16)    # xbar^T [ (c i j), b ]
            nc.vector.tensor_copy(xbar[:], xbar32[:])
            xbar_v = xbar[:].rearrange("p (kt b) -> p kt b", kt=6)

            # ---------- phase B: base forward (H-sliced, fp16 matmuls) ------
            wp_v = wp_sb[:].rearrange("p (k m) -> p k m", k=6)
            F_sb = sb.tile([128, 6 * 32], F16)   # f^T
            for m in range(6):
                pf = pst()
                for k in range(6):
                    nc.tensor.matmul(pf[:], wp_v[:, k, 128 * m:128 * (m + 1)],
                                     xbar_v[:, k, :], start=(k == 0), stop=False)
                nc.tensor.matmul(pf[:], bpr_sb[0:1, 128 * m:128 * (m + 1)],
                                 ones_sb[0:1, :], start=False, stop=True)
                nc.scalar.copy(F_sb[:, m * 32:(m + 1) * 32], pf[:])
            F_v = F_sb[:].rearrange("p (k b) -> p k b", k=6)

            w1_v = w1_sb[:].rearrange("p (k m) -> p k m", k=6)
            a_sb = sb.tile([128, 3 * 32], F16)
            mask_sb = sb.tile([128, 3 * 32], F32)
            for m in range(3):
                pz = pst()
                for k in range(6):
                    nc.tensor.matmul(pz[:], w1_v[:, k, 128 * m:128 * (m + 1)],
                                     F_v[:, k, :], start=(k == 0), stop=False)
                nc.tensor.matmul(pz[:], b1r_sb[0:1, 128 * m:128 * (m + 1)],
                                 ones_sb[0:1, :], start=False, stop=True)
                nc.vector.tensor_scalar(a_sb[:, m * 32:(m + 1) * 32], pz[:],
                                        0.0, None, op0=mybir.AluOpType.max)
                nc.vector.tensor_scalar(mask_sb[:, m * 32:(m + 1) * 32], pz[:],
                                        0.0, None, op0=mybir.AluOpType.is_gt)
            a_v = a_sb[:].rearrange("p (k b) -> p k b", k=3)

            w2_v = w2_sb[:].rearrange("p (k m) -> p k m", k=3)
            basep_sb = sb.tile([128, 6 * 32], F16)   # partial base^T (no b2)
            for m in range(6):
                pb = pst()
                for k in range(3):
                    nc.tensor.matmul(pb[:], w2_v[:, k, 128 * m:128 * (m + 1)],
                                     a_v[:, k, :], start=(k == 0), stop=(k == 2))
                nc.scalar.copy(basep_sb[:, m * 32:(m + 1) * 32], pb[:])
            basep_v = basep_sb[:].rearrange("p (k b) -> p k b", k=6)

            # metanet pre-activation partial: mW1^T @ basep  [192, 32]
            mw1_v = mw1_sb[:].rearrange("p (k m) -> p k m", k=6)
            m1p0 = sb.tile([128, 32], F32)
            m1p1 = sb.tile([64, 32], F32)
            for mi, (mp, msl) in enumerate(((m1p0, slice(0, 128)),
                                            (m1p1, slice(128, 192)))):
                pm = pst(128 if mi == 0 else 64)
                for k in range(6):
                    nc.tensor.matmul(pm[:], mw1_v[:, k, msl], basep_v[:, k, :],
                                     start=(k == 0), stop=(k == 5))
                nc.scalar.copy(mp[:], pm[:])

            # metanet constant: mW1^T @ b2 + mb1  [192, 1]
            mc0 = sb.tile([128, 1], F32)
            mc1 = sb.tile([64, 1], F32)
            for mi, (mp, msl) in enumerate(((mc0, slice(0, 128)),
                                            (mc1, slice(128, 192)))):
                pm = ps.tile([128 if mi == 0 else 64, 1], F32, tag="ps",
                             name="pmc")
                for k in range(6):
                    nc.tensor.matmul(pm[:], mw1_v[:, k, msl], b2t_sb[:, k:k + 1],
                                     start=(k == 0), stop=False)
                nc.tensor.matmul(pm[:], mb1r_sb[0:1, msl], ones_sb[0:1, 0:1],
                                 start=False, stop=True)
                nc.scalar.copy(mp[:], pm[:])

            arm_in = dr.tile([MH, 32], F32)
            arm_out = dr.tile([MH, 32], F32)
            nc.sync.dma_start(arm_in[0:128, :], m1p0[:])
            nc.sync.dma_start(arm_in[128:192, :], m1p1[:])
            nc.gpsimd.collective_compute(
                "AllReduce", ADD, replica_groups=RG,
                ins=[arm_in[:].opt()], outs=[arm_out[:].opt()])
            m1s0 = sb.tile([128, 32], F32)
            m1s1 = sb.tile([64, 32], F32)
            nc.sync.dma_start(m1s0[:], arm_out[0:128, :])
            nc.sync.dma_start(m1s1[:], arm_out[128:192, :])
            m1a = sb.tile([128, 32], F16)
            m1b = sb.tile([64, 32], F16)
            nc.vector.tensor_scalar(m1a[:], m1s0[:], mc0[:], 0.0,
                                    op0=ADD, op1=mybir.AluOpType.max)
            nc.vector.tensor_scalar(m1b[:], m1s1[:], mc1[:], 0.0,
                                    op0=ADD, op1=mybir.AluOpType.max)

            # coefs cT' [48, 32], rows = p-block (order _PORDER) * 8 + t
            pc = pst(48)
            nc.tensor.matmul(pc[:], mw2_sb[:, 0:48], m1a[:],
                             start=True, stop=False)
            nc.tensor.matmul(pc[:], mw2_sb[0:64, 48:96], m1b[:],
                             start=False, stop=False)
            nc.tensor.matmul(pc[:], mb2p_sb[0:1, :], ones_sb[0:1, :],
                             start=False, stop=True)
            cT = sb.tile([48, 32], F32)
            nc.scalar.copy(cT[:], pc[:])

            # replicate scale rows (first 24) across 128 partitions via DRAM
            cdram = dr.tile([48, 32], F32)
            nc.sync.dma_start(cdram[:], cT[:])
            crep = sb.tile([128, 24 * 32], F32)
            nc.sync.dma_start(
                crep[:].rearrange("p (r b) -> p r b", r=24),
                cdram[0:24, :].unsqueeze(0).partition_broadcast(128))
            crep_v = crep[:].rearrange("p (pb t b) -> p pb t b", pb=3, t=8)
            # bias coefficient rows, re-landed at partition 0 for matmul rhs
            cb1f = sb.tile([T, 32], F32)
            cb3f = sb.tile([T, 32], F32)
            cb5f = sb.tile([T, 32], F32)
            nc.sync.dma_start(cb1f[:], cdram[24:32, :])
            nc.sync.dma_start(cb3f[:], cdram[32:40, :])
            nc.sync.dma_start(cb5f[:], cdram[40:48, :])
            cb1 = sb.tile([T, 32], F16)
            cb3 = sb.tile([T, 32], F16)
            cb5 = sb.tile([T, 32], F16)
            nc.vector.tensor_copy(cb1[:], cb1f[:])
            nc.vector.tensor_copy(cb3[:], cb3f[:])
            nc.vector.tensor_copy(cb5[:], cb5f[:])

            # ---------- phase C: per-task scaled rhs copies (fp16) ----------
            xts = sb.tile([128, T * 6 * 32], F16)
            nc.vector.tensor_tensor(
                xts[:].rearrange("p (t k b) -> p t k b", t=T, k=6),
                xbar_v.unsqueeze(1).broadcast_to([128, T, 6, 32]),
                crep_v[:, 0].unsqueeze(2).broadcast_to([128, T, 6, 32]),
                op=MULT)
            xts_v = xts[:].rearrange("p (t k b) -> p t k b", t=T, k=6)

            fts = sb.tile([128, T * 6 * 32], F16)
            nc.vector.tensor_tensor(
                fts[:].rearrange("p (t k b) -> p t k b", t=T, k=6),
                F_v.unsqueeze(1).broadcast_to([128, T, 6, 32]),
                crep_v[:, 1].unsqueeze(2).broadcast_to([128, T, 6, 32]),
                op=MULT)
            fts_v = fts[:].rearrange("p (t k b) -> p t k b", t=T, k=6)

            ats = sb.tile([128, T * 3 * 32], F16)
            nc.vector.tensor_tensor(
                ats[:].rearrange("p (t k b) -> p t k b", t=T, k=3),
                a_v.unsqueeze(1).broadcast_to([128, T, 3, 32]),
                crep_v[:, 2].unsqueeze(2).broadcast_to([128, T, 3, 32]),
                op=MULT)
            ats_v = ats[:].rearrange("p (t k b) -> p t k b", t=T, k=3)

            # ---------- phase D: delta matmuls (fp16) ----------
            # df chunk [96, 32]
            dwp_v = dwp_sb[:].rearrange("p (tk m) -> p tk m", tk=48)
            pdf = pst(DS)
            for t in range(T):
                for k in range(6):
                    nc.tensor.matmul(pdf[:], dwp_v[:, t * 6 + k, :],
                                     xts_v[:, t, k, :],
                                     start=(t == 0 and k == 0), stop=False)
            nc.tensor.matmul(pdf[:], dbps_sb[:], cb1[:],
                             start=False, stop=True)
            df_sb = sb.tile([DS, 32], F32)
            nc.scalar.copy(df_sb[:], pdf[:])

            agd_in = dr.tile([DS, 32], F32)
            agd_out = dr.tile([D, 32], F32)
            nc.sync.dma_start(agd_in[:], df_sb[:])
            nc.gpsimd.collective_compute(
                "AllGather", BYP, replica_groups=RG,
                ins=[agd_in[:].opt()], outs=[agd_out[:].opt()])
            dfT32 = sb.tile([128, 6 * 32], F32)
            nc.sync.dma_start(
                dfT32[:].rearrange("p (k b) -> p k b", k=6),
                agd_out[:, :].rearrange("(k p) b -> p k b", k=6, p=128))
            dfT = sb.tile([128, 6 * 32], F16)
            nc.vector.tensor_copy(dfT[:], dfT32[:])
            dfT_v = dfT[:].rearrange("p (k b) -> p k b", k=6)

            # S_Q slice [384, 32]: sum_t dW1[t][:, hs]^T @ (c2-scaled f^T)
            dw1_v = dw1_sb[:].rearrange("p (tk m) -> p tk m", tk=48)
            psQ = [pst() for _ in range(3)]
            for tk in range(48):
                t, k = tk // 6, tk % 6
                for m in range(3):
                    nc.tensor.matmul(psQ[m][:],
                                     dw1_v[:, tk, 128 * m:128 * (m + 1)],
                                     fts_v[:, t, k, :],
                                     start=(tk == 0), stop=False)
            sq_sb = sb.tile([128, 3 * 32], F32)
            for m in range(3):
                nc.tensor.matmul(psQ[m][:], db1s_sb[:, 128 * m:128 * (m + 1)],
                                 cb3[:], start=False, stop=True)
                nc.scalar.copy(sq_sb[:, m * 32:(m + 1) * 32], psQ[m][:])
            sq_v = sq_sb[:].rearrange("p (k b) -> p k b", k=3)

            # R partial [768, 32]: sum_t dW2[t][hs, :]^T @ (c4-scaled a^T)
            dw2_v = dw2_sb[:].rearrange("p (tk m) -> p tk m", tk=24)
            psR = [pst() for _ in range(6)]
            for tk in range(24):
                t, k = tk // 3, tk % 3
                for m in range(6):
                    nc.tensor.matmul(psR[m][:],
                                     dw2_v[:, tk, 128 * m:128 * (m + 1)],
                                     ats_v[:, t, k, :],
                                     start=(tk == 0), stop=(tk == 23))
            R_sb = sb.tile([128, 6 * 32], F32)
            for m in range(6):
                nc.scalar.copy(R_sb[:, m * 32:(m + 1) * 32], psR[m][:])
            R_v = R_sb[:].rearrange("p (k b) -> p k b", k=6)

            # ---------- phase E: tail ----------
            da_sb = sb.tile([128, 3 * 32], F16)
            tmp_sb = sb.tile([128, 3 * 32], F32)
            for m in range(3):
                pz = pst()
                for k in range(6):
                    nc.tensor.matmul(pz[:], w1_v[:, k, 128 * m:128 * (m + 1)],
                                     dfT_v[:, k, :], start=(k == 0),
                                     stop=(k == 5))
                nc.vector.tensor_tensor(tmp_sb[:, m * 32:(m + 1) * 32], pz[:],
                                        sq_v[:, m, :], op=ADD)
                nc.vector.tensor_tensor(da_sb[:, m * 32:(m + 1) * 32],
                                        tmp_sb[:, m * 32:(m + 1) * 32],
                                        mask_sb[:, m * 32:(m + 1) * 32],
                                        op=MULT)
            da_v = da_sb[:].rearrange("p (k b) -> p k b", k=3)

            contrib = sb.tile([128, 6 * 32], F32)
            for m in range(6):
                po = pst()
                for k in range(3):
                    nc.tensor.matmul(po[:], w2_v[:, k, 128 * m:128 * (m + 1)],
                                     da_v[:, k, :], start=(k == 0),
                                     stop=(k == 2))
                nc.vector.tensor_tensor(tmp_sb[:, 0:32], po[:],
                                        R_v[:, m, :], op=ADD)
                nc.vector.tensor_tensor(contrib[:, m * 32:(m + 1) * 32],
                                        tmp_sb[:, 0:32],
                                        basep_v[:, m, :], op=ADD)

            # db2 bias term (local, added post-ReduceScatter)
            pb2 = pst(DS)
            nc.tensor.matmul(pb2[:], db2c_sb[:], cb5[:],
                             start=True, stop=True)
            b2term = sb.tile([DS, 32], F32)
            nc.vector.tensor_scalar(b2term[:], pb2[:], b2cc_sb[:], None,
                                    op0=ADD)

            rs_in = dr.tile([D, 32], F32)
            rs_out = dr.tile([DS, 32], F32)
            nc.sync.dma_start(
                rs_in[:].rearrange("(k p) b -> p k b", k=6, p=128),
                contrib[:].rearrange("p (k b) -> p k b", k=6))
            nc.gpsimd.collective_compute(
                "ReduceScatter", ADD, replica_groups=RG,
                ins=[rs_in[:].opt()], outs=[rs_out[:].opt()])
            fin = sb.tile([DS, 32], F32)
            nc.sync.dma_start(fin[:], rs_out[:, :])
            out_sb = sb.tile([DS, 32], F32)
            nc.vector.tensor_tensor(out_sb[:], fin[:], b2term[:], op=ADD)
            nc.sync.dma_start(out[:, :], out_sb[:])

    nc.compile()
    return nc


_NC_CACHE = None


def _get_nc():
    global _NC_CACHE
    if _NC_CACHE is None:
        _NC_CACHE = _build_nc()
    return _NC_CACHE


_RUN_CACHE = None


def _get_runner():
    """Mirror of bass2jax.run_bass_via_pjrt's multi-core path, but inputs are
    device_put + block_until_ready'ed BEFORE the execute call so all 8 cores
    start with data resident (minimizes the NEFF-start skew barrier)."""
    global _RUN_CACHE
    if _RUN_CACHE is not None:
        return _RUN_CACHE
    import jax
    from jax.sharding import Mesh, PartitionSpec, NamedSharding
    from jax.experimental.shard_map import shard_map
    from concourse import bass2jax, mybir as _mybir

    nc = _get_nc()
    bass2jax.install_neuronx_cc_hook()

    in_names, out_names, out_avals, zero_shapes = [], [], [], []
    partition_name = (nc.partition_id_tensor.name
                      if nc.partition_id_tensor else None)
    for alloc in nc.m.functions[0].allocations:
        if not isinstance(alloc, _mybir.MemoryLocationSet):
            continue
        name = alloc.memorylocations[0].name
        if alloc.kind == "ExternalInput":
            if name != partition_name:
                in_names.append(name)
        elif alloc.kind == "ExternalOutput":
            shape = tuple(alloc.tensor_shape)
            dtype = _mybir.dt.np(alloc.dtype)
            out_names.append(name)
            out_avals.append(jax.core.ShapedArray(shape, dtype))
            zero_shapes.append((shape, dtype))
    n_params = len(in_names)
    n_outs = len(out_avals)
    all_in_names = list(in_names) + list(out_names)
    if partition_name is not None:
        all_in_names.append(partition_name)

    def _body(*args):
        operands = list(args)
        if partition_name is not None:
            operands.append(bass2jax.partition_id_tensor())
        outs = bass2jax._bass_exec_p.bind(
            *operands,
            out_avals=tuple(out_avals),
            in_names=tuple(all_in_names),
            out_names=tuple(out_names),
            lowering_input_output_aliases=(),
            sim_require_finite=True,
            sim_require_nnan=True,
            nc=nc,
        )
        return tuple(outs)

    devices = jax.devices()[:NCORES]
    mesh = Mesh(np.asarray(devices), ("core",))
    in_specs = (PartitionSpec("core"),) * (n_params + n_outs)
    out_specs = (PartitionSpec("core"),) * len(out_names)
    donate = tuple(range(n_params, n_params + n_outs))
    sharded = jax.jit(
        shard_map(_body, mesh=mesh, in_specs=in_specs, out_specs=out_specs,
                  check_rep=False),
        donate_argnums=donate, keep_unused=True)
    sh = NamedSharding(mesh, PartitionSpec("core"))

    def run(in_maps):
        per_core = [[np.asarray(m[name]) for name in in_names]
                    for m in in_maps]
        concat_in = [
            jax.device_put(
                np.concatenate([per_core[c][i] for c in range(NCORES)],
                               axis=0), sh)
            for i in range(n_params)]
        concat_zeros = [
            jax.device_put(
                np.zeros((NCORES * s[0], *s[1:]), dt), sh)
            for (s, dt) in zero_shapes]
        jax.block_until_ready(concat_in)
        jax.block_until_ready(concat_zeros)
        out_arrs = sharded(*concat_in, *concat_zeros)
        out_arrs = jax.block_until_ready(out_arrs)
        return [
            {name: np.asarray(out_arrs[i]).reshape(
                NCORES, *out_avals[i].shape)[c]
             for i, name in enumerate(out_names)}
            for c in range(NCORES)
        ]

    _RUN_CACHE = run
    return run


def _make_in_maps(x, Wp, bp, W1, b1, W2, b2,
                  dWp, dbp, dW1, db1, dW2, db2,
                  mW1, mb1, mW2, mb2):
    x = np.asarray(x, dtype=np.float32)
    f32 = lambda a: np.ascontiguousarray(np.asarray(a), dtype=np.float32)
    Wp, bp, W1, b1, W2, b2 = map(f32, (Wp, bp, W1, b1, W2, b2))
    dWp, dbp, dW1, db1, dW2, db2 = map(f32, (dWp, dbp, dW1, db1, dW2, db2))
    mW1, mb1, mW2, mb2 = map(f32, (mW1, mb1, mW2, mb2))

    perm = _metanet_perm()
    mW2p = np.ascontiguousarray(mW2[:, perm])
    mb2p = np.ascontiguousarray(mb2[perm])[None, :]

    selA = np.zeros((126, 12), dtype=np.float32)
    for b in range(3):
        for c in range(3):
            for pi in range(14):
                selA[b * 42 + c * 14 + pi, c * 4 + b] = 1.0 / NP
    selB = np.zeros((42, 12), dtype=np.float32)
    for c in range(3):
        for pi in range(14):
            selB[c * 14 + pi, c * 4 + 3] = 1.0 / NP

    ones = np.ones((1, 32), dtype=np.float32)
    b2t = np.ascontiguousarray(b2.reshape(6, 128).T)
    bsel_rows = []
    for i in range(NCORES):
        r = np.zeros((128, B), dtype=np.float32)
        r[:, BL * i:BL * (i + 1)] = 1.0
        bsel_rows.append(r)

    in_maps = []
    for i in range(NCORES):
        hs = slice(HS * i, HS * (i + 1))
        dsl = slice(DS * i, DS * (i + 1))
        m = {
            "xs": np.ascontiguousarray(x[BL * i:BL * (i + 1)]).reshape(168, 3584),
            "selA": selA, "selB": selB, "ones": ones,
            "Wp": Wp, "bpr": bp[None, :],
            "W1s": np.ascontiguousarray(W1[:, hs]), "b1r": b1[None, hs],
            "W2s": np.ascontiguousarray(W2[hs, :]),
            "mW1": mW1, "mb1r": mb1[None, :],
            "mW2p": mW2p, "mb2p": mb2p,
            "b2t": b2t, "b2cc": b2[dsl, None],
            "dWps": np.ascontiguousarray(dWp[:, :, dsl]).reshape(T * D, DS),
            "dW1s": np.ascontiguousarray(dW1[:, :, hs]).reshape(T * D, HS),
            "dW2s": np.ascontiguousarray(dW2[:, hs, :]).reshape(T * HS, D),
            "dbps": np.ascontiguousarray(dbp[:, dsl]),
            "db1s": np.ascontiguousarray(db1[:, hs]),
            "db2c": np.ascontiguousarray(db2[:, dsl]),
            "bsel": bsel_rows[i],
        }
        in_maps.append(m)
    return in_maps


def _assemble(results):
    chunks = [results[i]["out"] for i in range(NCORES)]
    full = np.concatenate(chunks, axis=0)      # [768, 32]
    return np.ascontiguousarray(full.T).astype(np.float32)   # [32, 768]


def kernel(**inputs) -> np.ndarray:
    in_maps = _make_in_maps(**inputs)
    results = _get_runner()(in_maps)
    return _assemble(results)


def kernel_traced(**inputs):
    """Like kernel() but returns (output, exec_time_ns) via neuron-profile.

    Uses the same pre-staged runner as kernel(); wraps the execute call in
    the axon NTFF profiling hook (registered by the caller / test harness).
    """
    import tempfile
    from antenv.axon_hooks import get_axon_ntff_profile_hook
    import gauge.profiler
    from concourse._compat import FishPath
    from concourse.bass_utils import _process_ntff_profile

    in_maps = _make_in_maps(**inputs)
    run = _get_runner()
    # warm-up execution (compiles + caches the executable)
    run(in_maps)

    hook = get_axon_ntff_profile_hook()
    neff_dir = tempfile.mkdtemp()
    with hook(neff_dir, list(range(NCORES))):
        results = run(in_maps)

    profile = gauge.profiler.Profile(
        profile_path=FishPath(neff_dir),
        kernel_dev_mode=True, profile_on_exit=False,
        bass_kernel=_get_nc().m, offline_processing=True,
        fname="*_body*", metadata={})
    pr = _process_ntff_profile(profile, neff_dir, _get_nc(),
                               list(range(NCORES)), list(range(NCORES)),
                               False, {}, trace_events=False)
    return _assemble(results), pr.exec_time_ns
